# revision 1
# baseline (speedup 1.0000x reference)
"""Trainium2 Bass kernel for nn_DilatedAttention (B=2, L=4096, E=512, H=8, D=64,
dilation=2, window=256, causal, pre-norm transformer block with MLP).

Strategy
--------
* 8 cores, sequence-parallel: core c owns tokens [512c, 512c+512) of both
  batches.  The attention mask is local (|i-j| <= 256), so each core also
  computes K/V for a 256-token halo before its range (zero-padded for c=0).
* The dilation-2 + causal + window mask couples only equal-parity positions:
  even tokens attend even tokens, odd attend odd.  After de-interleaving by
  parity the mask is a plain causal sliding-window (window=128) attention over
  a length-2048 subsequence.  Parity de-interleave is free: activations are
  kept feature-major ([feature, token]) so parity is a stride-2 slice on the
  free axis.
* Feature-major layout everywhere: weights are used directly as matmul lhsT
  (stationary) operands, activations as the moving operand.  LayerNorm is
  decomposed as h = x*rstd + z with z = -mu*rstd; the rstd scale is broadcast
  across partitions with a rank-1 PE matmul (ones ⊗ rstd into PSUM) and the z
  term is folded into each downstream matmul as a rank-1 accumulation against
  host-precomputed weight column sums.  LN gain/bias fold into the weights on
  the host.
* Attention masking: scores are exponentiated unmasked; the sliding-window
  mask is applied by zeroing invalid probability entries with gpsimd
  affine_select (uniform across cores), and the core-0 halo padding is
  neutralized by a per-core "valid key" 0/1 column appended to V (which also
  produces the softmax denominator row).
* Matmul inputs in bf16 (fp32 PSUM accumulation); the residual stream stays
  fp32.
"""

import os
import sys
import types
import numpy as np
import ml_dtypes

import concourse.bass as bass
import concourse.mybir as mybir
import concourse.tile as tile
from concourse.bass_utils import run_bass_kernel_spmd
from concourse.masks import make_identity


def _install_ntff_hook_shim():
    """This image's antenv lacks axon_hooks; bass_utils imports it when
    BASS_TRACE is set.  Provide the ctypes-based NTFF hook (or a None hook)
    so tracing works — and never crashes — in any environment."""
    try:
        import antenv
    except ImportError:
        return
    try:
        from antenv.axon_hooks import get_axon_ntff_profile_hook  # noqa: F401
        return  # real module present
    except ImportError:
        pass
    import ctypes
    import contextlib

    hook = None
    so_path = "/opt/axon/libaxon_pjrt.so"
    if os.path.exists(so_path):
        try:
            lib = ctypes.CDLL(so_path)
            if hasattr(lib, "axon_start_nrt_profile"):
                lib.axon_start_nrt_profile.argtypes = [
                    ctypes.POINTER(ctypes.c_int64), ctypes.c_size_t]
                lib.axon_start_nrt_profile.restype = ctypes.c_int64
                lib.axon_stop_nrt_profile.argtypes = [ctypes.c_char_p]
                lib.axon_stop_nrt_profile.restype = ctypes.c_int64

                @contextlib.contextmanager
                def _hook(output_dir, device_ids):
                    import jax
                    jax.devices()
                    if device_ids:
                        ids = (ctypes.c_int64 * len(device_ids))(*device_ids)
                        rc = lib.axon_start_nrt_profile(ids, len(device_ids))
                    else:
                        rc = lib.axon_start_nrt_profile(None, 0)
                    if rc != 0:
                        raise RuntimeError(f"axon_start_nrt_profile rc={rc}")
                    try:
                        yield
                    finally:
                        lib.axon_stop_nrt_profile(str(output_dir).encode())

                hook = _hook
        except OSError:
            hook = None

    mod = types.ModuleType("antenv.axon_hooks")
    mod.get_axon_ntff_profile_hook = lambda: hook
    mod.set_axon_ntff_profile_hook = lambda h: None
    sys.modules["antenv.axon_hooks"] = mod
    antenv.axon_hooks = mod


_install_ntff_hook_shim()

F32 = mybir.dt.float32
BF16 = mybir.dt.bfloat16
AF = mybir.ActivationFunctionType
ALU = mybir.AluOpType

# problem constants
B, L, E, H, D = 2, 4096, 512, 8, 64
HID = 2048
EPS = 1e-5
WIN, DIL = 256, 2
N_CORES = 8
S = L // N_CORES          # tokens per core per batch (512)
HALO = WIN                # kv halo tokens (256)
T_EXT = S + HALO          # 768
EC = E // 128             # 4 feature chunks
HC = HID // 128           # 16 hidden chunks
NQ = S // 2               # queries per parity (256)
KB = (NQ + 128) // 128    # key blocks per parity (3)
QB = NQ // 128            # query blocks per parity (2)


def _legalize_waits(m, max_waits=1):
    """The walrus build here accepts only one sync-wait command per lowered
    instruction; hoist extras onto same-engine NoOps placed just before."""
    for fn in m.functions:
        for blk in fn.blocks:
            new_list = []
            for ins in blk.instructions:
                si = ins.sync_info
                if si is not None and si.on_wait is not None and len(si.on_wait) > max_waits:
                    waits = list(si.on_wait)
                    extra, keep = waits[:-max_waits], waits[-max_waits:]
                    k = 0
                    while extra:
                        chunk, extra = extra[:max_waits], extra[max_waits:]
                        nop = mybir.InstNoOp(name=f"{ins.name}-wsplit{k}", ins=[], outs=[])
                        nop.engine = ins.engine
                        nop.sync_info = mybir.SyncInfo(on_wait=chunk, on_update=[])
                        new_list.append(nop)
                        k += 1
                    si.on_wait = keep
                new_list.append(ins)
            blk.instructions = new_list


def build_program(has_qk_bias: bool, has_v_bias: bool, has_out_bias: bool, has_b2: bool):
    nc = bass.Bass("TRN2", target_bir_lowering=False, debug=False)

    # ---- DRAM I/O ----
    xT = nc.dram_tensor("xT", [B, E, T_EXT], F32, kind="ExternalInput").ap()
    wqkv = nc.dram_tensor("wqkv", [E, 3 * E], BF16, kind="ExternalInput").ap()
    wout = nc.dram_tensor("wout", [E, E], BF16, kind="ExternalInput").ap()
    w1 = nc.dram_tensor("w1", [E, HID], BF16, kind="ExternalInput").ap()
    w2 = nc.dram_tensor("w2", [HID, E], BF16, kind="ExternalInput").ap()
    vones_in = nc.dram_tensor("vones", [KB * 128], BF16, kind="ExternalInput").ap()
    vmlp_in = nc.dram_tensor("vmlp", [HID], F32, kind="ExternalInput").ap()
    urow_in = nc.dram_tensor("urow", [3 * E], BF16, kind="ExternalInput").ap()
    u2row_in = nc.dram_tensor("u2row", [HID], BF16, kind="ExternalInput").ap()
    if has_qk_bias:
        vqk_in = nc.dram_tensor("vqk", [2 * E], F32, kind="ExternalInput").ap()
    if has_v_bias:
        vvb_in = nc.dram_tensor("vvb", [E], F32, kind="ExternalInput").ap()
    if has_out_bias:
        outb_in = nc.dram_tensor("outb", [E], F32, kind="ExternalInput").ap()
    if has_b2:
        b2_in = nc.dram_tensor("b2v", [E], F32, kind="ExternalInput").ap()
    yT = nc.dram_tensor("yT", [B, E, S], F32, kind="ExternalOutput").ap()

    with tile.TileContext(nc) as tc:
        ctxstack = []

        def pool(name, bufs, space="SBUF"):
            p = tc.tile_pool(name=name, bufs=bufs, space=space)
            ctxstack.append(p)
            return p.__enter__()

        wpool = pool("wpool", 1)
        xpool = pool("xpool", 2)
        x1pool = pool("x1pool", 2)
        qkpool = pool("qkpool", 1)
        vpool = pool("vpool", 2)
        ptpool = pool("ptpool", 4)
        bbpool = pool("bbpool", 4)
        opool = pool("opool", 1)
        x2pool = pool("x2pool", 2)
        hpool = pool("hpool", 1)
        ypool = pool("ypool", 2)
        stpool = pool("stpool", 2)
        sqpool = pool("sqpool", 2)
        rpool = pool("rpool", 4)

        pstat = pool("pstat", 1, space="PSUM")
        pmain = pool("pmain", 2, space="PSUM")
        patt = pool("patt", 5, space="PSUM")
        psbp = patt

        # ---- x for both batches first (critical path), then qkv weights ----
        xts = []
        for b in range(B):
            xt = xpool.tile([128, EC, T_EXT], F32, tag="xt", name=f"xt{b}")
            nc.sync.dma_start(xt, xT[b].rearrange("(c p) t -> p c t", p=128))
            xts.append(xt)
        wqkv_sb = wpool.tile([128, EC, 3 * E], BF16)
        nc.sync.dma_start(wqkv_sb, wqkv.rearrange("(c p) f -> p c f", p=128))
        # small constants on the sync queue
        vmlp_sb = wpool.tile([128, HC], F32)
        nc.sync.dma_start(vmlp_sb, vmlp_in.rearrange("(s p) -> p s", p=128))
        urow_sb = wpool.tile([1, 3 * E], BF16)
        nc.sync.dma_start(urow_sb, urow_in[None, :])
        u2row_sb = wpool.tile([1, HID], BF16)
        nc.sync.dma_start(u2row_sb, u2row_in[None, :])
        vones_sb = wpool.tile([128, KB], BF16)
        nc.sync.dma_start(vones_sb, vones_in.rearrange("(k p) -> p k", p=128))
        if has_qk_bias:
            vqk_sb = wpool.tile([128, 8], F32)
            nc.sync.dma_start(vqk_sb, vqk_in.rearrange("(s p) -> p s", p=128))
        if has_v_bias:
            vvb_sb = wpool.tile([128, E], F32)
            nc.sync.dma_start(vvb_sb, vvb_in[None, :].to_broadcast([128, E]))
        if has_out_bias:
            outb_sb = wpool.tile([128, EC], F32)
            nc.sync.dma_start(outb_sb, outb_in.rearrange("(s p) -> p s", p=128))
        if has_b2:
            b2_sb = wpool.tile([128, EC], F32)
            nc.sync.dma_start(b2_sb, b2_in.rearrange("(s p) -> p s", p=128))
        # later-needed weights on the second (Activation) HWDGE queue
        wout_sb = wpool.tile([128, EC, E], BF16)
        nc.scalar.dma_start(wout_sb, wout.rearrange("(c p) f -> p c f", p=128))
        w1_sb = wpool.tile([128, EC, HID], BF16)
        nc.scalar.dma_start(w1_sb, w1.rearrange("(c p) f -> p c f", p=128))
        w2_sb = wpool.tile([128, HC, E], BF16)
        nc.scalar.dma_start(w2_sb, w2.rearrange("(c p) f -> p c f", p=128))

        ident = wpool.tile([128, 128], BF16)
        make_identity(nc, ident)
        ones_row = wpool.tile([1, 128], BF16)
        nc.vector.memset(ones_row, 1.0)
        ones_col = wpool.tile([128, 1], BF16)
        nc.vector.memset(ones_col, 1.0)
        eps_t = wpool.tile([1, 1], F32)
        nc.vector.memset(eps_t, EPS)

        def layernorm_stats(xt_ap, T):
            """xt_ap: [128, EC, T] fp32 feature-major slab.
            Returns (rstd_bf [1,T] bf16, zrow [1,T] bf16 = -mu*rstd)."""
            n_tt = (T + 383) // 384
            tts = [(i * T // n_tt, (i + 1) * T // n_tt) for i in range(n_tt)]
            mu_full = stpool.tile([1, T_EXT], F32, tag="mu", name="mu")
            tmp_full = stpool.tile([1, T_EXT], F32, tag="sttmp", name="sttmp")
            rstd_full = stpool.tile([1, T_EXT], F32, tag="strstd", name="strstd")
            mu_neg, tmp, rstd = mu_full[:, :T], tmp_full[:, :T], rstd_full[:, :T]
            for (t0, t1) in tts:
                # bf16 shadow of x for the ones-matmul stats (~1e-4 on mu/rstd)
                xbf_full = sqpool.tile([128, EC, 384], BF16, tag="xbf", name="xbf")
                xbf = xbf_full[:, :, : t1 - t0]
                for c in range(EC):
                    nc.scalar.copy(xbf[:, c, :], xt_ap[:, c, t0:t1])
                ps_s = pstat.tile([1, t1 - t0], F32, tag="pstat")
                for c in range(EC):
                    nc.tensor.matmul(ps_s, lhsT=ones_col, rhs=xbf[:, c, :],
                                     start=(c == 0), stop=(c == EC - 1))
                nc.scalar.mul(mu_neg[:, t0:t1], ps_s, -1.0 / E)
                ps_q = pstat.tile([1, t1 - t0], F32, tag="pstat")
                for c in range(EC):
                    xsq_full = sqpool.tile([128, 512], BF16, tag="xsq", name="xsq")
                    xsq = xsq_full[:, : t1 - t0]
                    nc.scalar.square(xsq, xt_ap[:, c, t0:t1])
                    nc.tensor.matmul(ps_q, lhsT=ones_col, rhs=xsq,
                                     start=(c == 0), stop=(c == EC - 1))
                nc.scalar.mul(tmp[:, t0:t1], ps_q, 1.0 / E)
            # var = E[x^2] - mu^2 ; rstd = 1/sqrt(var+eps) ; z = -mu*rstd
            musq = rstd  # scratch: overwritten by the reciprocal at the end
            nc.scalar.square(musq, mu_neg)
            nc.vector.tensor_tensor(tmp, tmp, musq, ALU.subtract)
            nc.scalar.activation(tmp, tmp, AF.Sqrt, bias=eps_t)
            nc.vector.reciprocal(rstd, tmp)
            zrow = stpool.tile([1, T_EXT], BF16, tag="stz", name="stz")[:, :T]
            rstd_bf = stpool.tile([1, T_EXT], BF16, tag="strbf", name="strbf")[:, :T]
            nc.vector.tensor_copy(rstd_bf, rstd)
            nc.vector.tensor_tensor(zrow, mu_neg, rstd_bf, ALU.mult)
            return rstd_bf, zrow

        # ---- stage A (both batches up front): LN1 stats + x1 = x*rstd ----
        x1s, zrows = [], []
        for b in range(B):
            xt = xts[b]
            rstd_bf, zrow = layernorm_stats(xt, T_EXT)
            x1 = x1pool.tile([128, EC, T_EXT], BF16, tag="x1", name=f"x1_{b}")
            for tt in range(2):
                t0, t1 = tt * 384, (tt + 1) * 384
                ps_b = psbp.tile([128, 384], F32, tag="patt")
                nc.tensor.matmul(ps_b, lhsT=ones_row, rhs=rstd_bf[:, t0:t1],
                                 start=True, stop=True)
                ps_z = psbp.tile([128, 384], F32, tag="patt", name="ps_z")
                nc.tensor.matmul(ps_z, lhsT=ones_row, rhs=zrow[:, t0:t1],
                                 start=True, stop=True)
                for c in range(EC):
                    nc.vector.tensor_tensor(x1[:, c, t0:t1], xt[:, c, t0:t1], ps_b, ALU.mult)
                    nc.vector.tensor_tensor(x1[:, c, t0:t1], x1[:, c, t0:t1], ps_z, ALU.add)
            x1s.append(x1)
            zrows.append(zrow)

        x2Ts = []
        for b in range(B):
            xt = xts[b]
            x1, zrow = x1s[b], zrows[b]
            x1_par = x1.rearrange("p c (t two) -> p c two t", two=2)
            zrow_par = zrow.rearrange("o (t two) -> o two t", two=2)

            # ---- stage B: QKV ----
            qkT = qkpool.tile([128, 8, T_EXT], BF16, tag="qkT")
            for fs in range(8):
                # queries only need the core's own tokens; keys need the halo
                spans = [(HALO, T_EXT)] if fs < 4 else [(0, 384), (384, T_EXT)]
                for t0, t1 in spans:
                    ps_full = pmain.tile([128, 512], F32, tag="pmain", name="ps_full")
                    ps = ps_full[:, : t1 - t0]
                    for c in range(EC):
                        nc.tensor.matmul(ps, lhsT=wqkv_sb[:, c, fs * 128:(fs + 1) * 128],
                                         rhs=x1[:, c, t0:t1],
                                         start=(c == 0), stop=(c == EC - 1))
                    if has_qk_bias:
                        nc.vector.tensor_scalar(qkT[:, fs, t0:t1], ps,
                                                vqk_sb[:, fs:fs + 1], None, ALU.add) \
                            if fs >= 4 else \
                            nc.scalar.activation(qkT[:, fs, t0:t1], ps, AF.Identity,
                                                 bias=vqk_sb[:, fs:fs + 1])
                    elif fs >= 4:
                        nc.vector.tensor_copy(qkT[:, fs, t0:t1], ps)
                    else:
                        nc.scalar.copy(qkT[:, fs, t0:t1], ps)
            qkT_par = qkT.rearrange("p s (t two) -> p s two t", two=2)

            # V token-major, parity-separated, with the valid-key column appended
            vplus = [vpool.tile([128, KB, H, D + 1], BF16, tag=f"vplus{p}", name=f"vplus{p}")
                     for p in range(2)]
            for par in range(2):
                nc.gpsimd.tensor_copy(vplus[par][:, :, :, D],
                                      vones_sb[:, :, None].to_broadcast([128, KB, H]))
                for kb in range(KB):
                    ps = pmain.tile([128, E], F32, tag="pmain")
                    for c in range(EC):
                        nc.tensor.matmul(ps, lhsT=x1_par[:, c, par, kb * 128:(kb + 1) * 128],
                                         rhs=wqkv_sb[:, c, 2 * E:3 * E],
                                         start=(c == 0), stop=(c == EC - 1))
                    pv = ps.rearrange("p (h d) -> p h d", h=H)
                    if has_v_bias:
                        nc.vector.tensor_tensor(vplus[par][:, kb, :, 0:D], pv,
                                                vvb_sb.rearrange("p (h d) -> p h d", h=H),
                                                ALU.add)
                    else:
                        nc.vector.tensor_copy(vplus[par][:, kb, :, 0:D], pv)

            # ---- stage C: attention (token-major O, per-partition denom) ----
            otT = opool.tile([128, EC, S], BF16, tag="otT")
            otT_par = otT.rearrange("p c (t two) -> p c two t", two=2)
            oslab = [opool.tile([128, QB, E], BF16, tag=f"oslab{p}", name=f"oslab{p}")
                     for p in range(2)]
            for par in range(2):
                for h in range(H):
                    rb = (h % 2) * 64
                    sl = h // 2
                    pt = ptpool.tile([128, KB, NQ], BF16, tag="pt")
                    for kb in range(KB):
                        ps_sc = patt.tile([128, NQ], F32, tag="patt")
                        nc.tensor.matmul(
                            ps_sc,
                            lhsT=qkT_par[rb:rb + 64, 4 + sl, par, kb * 128:(kb + 1) * 128],
                            rhs=qkT_par[rb:rb + 64, sl, par, 128:128 + NQ],
                            start=True, stop=True)
                        nc.scalar.activation(pt[:, kb, :], ps_sc, AF.Exp)
                        # sliding-window mask: zero invalid probabilities.
                        # keep iff 0 <= q - ((kb-1)*128 + k) <= 128
                        if kb < 2:   # key beyond (newer than) the query
                            nc.gpsimd.affine_select(
                                out=pt[:, kb, :], in_=pt[:, kb, :],
                                compare_op=ALU.is_ge, fill=0.0,
                                base=kb * 128, channel_multiplier=1,
                                pattern=[[-1, NQ]])
                        if kb > 0:   # key older than query-128 (outside window)
                            nc.gpsimd.affine_select(
                                out=pt[:, kb, :], in_=pt[:, kb, :],
                                compare_op=ALU.is_ge, fill=0.0,
                                base=-(kb - 1) * 128, channel_multiplier=-1,
                                pattern=[[1, NQ]])
                    for qb in range(QB):
                        ps_o = patt.tile([128, D + 1], F32, tag="patt")
                        for kb in range(KB):
                            nc.tensor.matmul(ps_o, lhsT=pt[:, kb, qb * 128:(qb + 1) * 128],
                                             rhs=vplus[par][:, kb, h, :],
                                             start=(kb == 0), stop=(kb == KB - 1))
                        rin = rpool.tile([128, 1], F32, tag="rin", name="rin")
                        nc.vector.reciprocal(rin, ps_o[:, D:D + 1])
                        nc.vector.tensor_scalar(oslab[par][:, qb, h * D:(h + 1) * D],
                                                ps_o[:, 0:D], rin, None, ALU.mult)
            # transpose O back to feature-major, re-interleaving parities
            for par in range(2):
                for qb in range(QB):
                    for fc in range(EC):
                        ps_t = patt.tile([128, 128], BF16, tag="patt")
                        nc.tensor.transpose(ps_t, oslab[par][:, qb, fc * 128:(fc + 1) * 128],
                                            ident)
                        nc.vector.tensor_copy(
                            otT_par[:, fc, par, qb * 128:(qb + 1) * 128], ps_t)

            # ---- stage D: out-proj + residual ----
            x2T = x2pool.tile([128, EC, S], F32, tag="x2T", name=f"x2T{b}")
            for es in range(EC):
                ps = pmain.tile([128, S], F32, tag="pmain")
                for c in range(EC):
                    nc.tensor.matmul(ps, lhsT=wout_sb[:, c, es * 128:(es + 1) * 128],
                                     rhs=otT[:, c, :], start=(c == 0), stop=(c == EC - 1))
                if has_out_bias:
                    nc.vector.tensor_scalar(ps, ps, outb_sb[:, es:es + 1], None, ALU.add)
                nc.vector.tensor_tensor(x2T[:, es, :], ps, xt[:, es, HALO:T_EXT], ALU.add)
            x2Ts.append(x2T)

        for b in range(B):
            x2T = x2Ts[b]
            # ---- stage E: LN2 ----
            rstd2_bf, z2row = layernorm_stats(x2T, S)
            x21 = x2pool.tile([128, EC, S], BF16, tag="x21")
            ps_b2 = psbp.tile([128, S], F32, tag="patt")
            nc.tensor.matmul(ps_b2, lhsT=ones_row, rhs=rstd2_bf, start=True, stop=True)
            ps_z2 = psbp.tile([128, S], F32, tag="patt", name="ps_z2")
            nc.tensor.matmul(ps_z2, lhsT=ones_row, rhs=z2row, start=True, stop=True)
            for c in range(EC):
                nc.vector.tensor_tensor(x21[:, c, :], x2T[:, c, :], ps_b2, ALU.mult)
                nc.vector.tensor_tensor(x21[:, c, :], x21[:, c, :], ps_z2, ALU.add)

            # ---- stage F: MLP ----
            h2T = hpool.tile([128, HC, S], BF16, tag="h2T")
            for hs in range(HC):
                ps = pmain.tile([128, S], F32, tag="pmain")
                for c in range(EC):
                    nc.tensor.matmul(ps, lhsT=w1_sb[:, c, hs * 128:(hs + 1) * 128],
                                     rhs=x21[:, c, :], start=(c == 0), stop=(c == EC - 1))
                nc.scalar.activation(h2T[:, hs, :], ps, AF.Gelu, bias=vmlp_sb[:, hs:hs + 1])
            for es in range(EC):
                ps = pmain.tile([128, S], F32, tag="pmain")
                for hc in range(HC):
                    nc.tensor.matmul(ps, lhsT=w2_sb[:, hc, es * 128:(es + 1) * 128],
                                     rhs=h2T[:, hc, :], start=(hc == 0), stop=(hc == HC - 1))
                if has_b2:
                    nc.vector.tensor_scalar(ps, ps, b2_sb[:, es:es + 1], None, ALU.add)
                yt = ypool.tile([128, S], F32, tag="yt", name="yt")
                nc.vector.tensor_tensor(yt, ps, x2T[:, es, :], ALU.add)
                nc.sync.dma_start(yT[b, es * 128:(es + 1) * 128, :], yt)

        for p in reversed(ctxstack):
            p.__exit__(None, None, None)

    return nc


_cached = {}


def _get_program(key):
    if key not in _cached:
        nc = build_program(*key)
        _legalize_waits(nc.m)
        _cached[key] = nc
    return _cached[key]


def _prepare_core_inputs(inputs):
    """Host-side folding + sharding. Returns (flags_key, in_maps list)."""
    x = np.asarray(inputs["x"], np.float32)
    ln1_g = np.asarray(inputs["ln1_g"], np.float32)
    ln1_b = np.asarray(inputs["ln1_b"], np.float32)
    qkv_w = np.asarray(inputs["qkv_w"], np.float32)
    qkv_b = np.asarray(inputs["qkv_b"], np.float32)
    out_w = np.asarray(inputs["out_w"], np.float32)
    out_b = np.asarray(inputs["out_b"], np.float32)
    ln2_g = np.asarray(inputs["ln2_g"], np.float32)
    ln2_b = np.asarray(inputs["ln2_b"], np.float32)
    w1 = np.asarray(inputs["w1"], np.float32)
    b1 = np.asarray(inputs["b1"], np.float32)
    w2 = np.asarray(inputs["w2"], np.float32)
    b2 = np.asarray(inputs["b2"], np.float32)

    # fold LN1 gain into qkv_w; fold attention 1/sqrt(D) into the Q part
    qscale = 1.0 / np.sqrt(D)
    wqkv_eff = ln1_g[:, None] * qkv_w
    vqkv = ln1_b @ qkv_w + qkv_b          # [3E]
    wqkv_eff[:, :E] *= qscale
    vqkv = vqkv.copy()
    vqkv[:E] *= qscale
    # fold LN2 gain into w1
    w1_eff = ln2_g[:, None] * w1
    vmlp = ln2_b @ w1 + b1                # [HID]

    has_qk_bias = bool(np.any(vqkv[: 2 * E] != 0.0))
    has_v_bias = bool(np.any(vqkv[2 * E:] != 0.0))
    has_out_bias = bool(np.any(out_b != 0.0))
    has_b2 = bool(np.any(b2 != 0.0))
    key = (has_qk_bias, has_v_bias, has_out_bias, has_b2)

    wqkv_bf = wqkv_eff.astype(ml_dtypes.bfloat16)
    wout_bf = out_w.astype(ml_dtypes.bfloat16)
    w1_bf = w1_eff.astype(ml_dtypes.bfloat16)
    w2_bf = w2.astype(ml_dtypes.bfloat16)
    urow_bf = wqkv_bf.astype(np.float32).sum(0).astype(ml_dtypes.bfloat16)
    u2row_bf = w1_bf.astype(np.float32).sum(0).astype(ml_dtypes.bfloat16)

    # x transposed per batch with halo: [B, E, T_EXT]
    xT_full = np.ascontiguousarray(x.transpose(0, 2, 1))  # [B, E, L]
    in_maps = []
    for c in range(N_CORES):
        s = c * S
        xTe = np.zeros((B, E, T_EXT), np.float32)
        lo = s - HALO
        src_lo = max(lo, 0)
        xTe[:, :, src_lo - lo:] = xT_full[:, :, src_lo:s + S]
        vones = np.ones(KB * 128, np.float32)
        if c == 0:
            vones[:128] = 0.0
        im = {
            "xT": xTe,
            "wqkv": wqkv_bf,
            "wout": wout_bf,
            "w1": w1_bf,
            "w2": w2_bf,
            "vones": vones.astype(ml_dtypes.bfloat16),
            "vmlp": vmlp.astype(np.float32),
            "urow": urow_bf,
            "u2row": u2row_bf,
        }
        if has_qk_bias:
            im["vqk"] = vqkv[: 2 * E].astype(np.float32)
        if has_v_bias:
            im["vvb"] = vqkv[2 * E:].astype(np.float32)
        if has_out_bias:
            im["outb"] = out_b.astype(np.float32)
        if has_b2:
            im["b2v"] = b2.astype(np.float32)
        in_maps.append(im)
    return key, in_maps


_last_results = None


def kernel(**inputs) -> np.ndarray:
    global _last_results
    key, in_maps = _prepare_core_inputs(inputs)
    nc = _get_program(key)
    res = run_bass_kernel_spmd(nc, in_maps, core_ids=list(range(N_CORES)))
    _last_results = res
    out = np.empty((B, L, E), np.float32)
    for c in range(N_CORES):
        yT = res.results[c]["yT"]          # [B, E, S]
        out[:, c * S:(c + 1) * S, :] = yT.transpose(0, 2, 1)
    return out



# revision 9
# speedup vs baseline: 1.3064x; 1.3064x over previous
"""Trainium2 Bass kernel for nn_DilatedAttention (B=2, L=4096, E=512, H=8, D=64,
dilation=2, window=256, causal, pre-norm transformer block with MLP).

Strategy (v2)
-------------
* 8 cores, sequence-parallel: core c owns tokens [512c, 512c+512) of both
  batches, with a 256-token K/V halo (zero-padded on core 0).
* Dilation-2 + causal + window couples only equal-parity tokens. The HOST
  packs tokens parity-major (all even tokens, then all odd) so every on-chip
  access is stride-1; the host un-packs the output. In parity space the mask
  is a causal sliding window of 128: for each 128-query block only the
  previous and the diagonal 128-key blocks matter, masked by two constant
  triangular 0/1 matrices (multiplied into the probabilities on DVE).
* Attention produces O directly feature-major: the O matmul uses V as the
  128-stationary ([128 keys, 64]) and the probabilities as moving, with PE
  column-tiling packing two heads per PSUM tile. Softmax denominators come
  from per-head one-hot rank-8 matmuls accumulated into one [8, 128] PSUM
  row-block, reciprocal on DVE, broadcast back with a constant selector
  matmul. No transposes anywhere.
* LayerNorm stats via ones[128,128] matmuls (broadcast across partitions for
  free); rstd = Exp(-0.5*Ln(var+eps)) on the Act engine so the whole kernel
  needs only the {ln,exp} and {gelu} activation tables (square/copy/identity
  are in every table set).
* All big GEMMs (QKV, out-proj, MLP) run in fp8e4 with DoubleRow perf mode
  (2 contraction tiles per pass). Weights are pre-scaled by 64 on the host
  (fp8 subnormal avoidance) and descaled in the PSUM->SBUF copies.
  Attention score/O matmuls stay bf16.
"""

import os
import sys
import types
import numpy as np
import ml_dtypes

import concourse.bass as bass
import concourse.mybir as mybir
import concourse.tile as tile
from concourse.bass_utils import run_bass_kernel_spmd


def _install_ntff_hook_shim():
    """This image's antenv lacks axon_hooks; bass_utils imports it when
    BASS_TRACE is set.  Provide the ctypes-based NTFF hook (or a None hook)
    so tracing works — and never crashes — in any environment."""
    try:
        import antenv
    except ImportError:
        return
    try:
        from antenv.axon_hooks import get_axon_ntff_profile_hook  # noqa: F401
        return  # real module present
    except ImportError:
        pass
    import ctypes
    import contextlib

    hook = None
    so_path = "/opt/axon/libaxon_pjrt.so"
    if os.path.exists(so_path):
        try:
            lib = ctypes.CDLL(so_path)
            if hasattr(lib, "axon_start_nrt_profile"):
                lib.axon_start_nrt_profile.argtypes = [
                    ctypes.POINTER(ctypes.c_int64), ctypes.c_size_t]
                lib.axon_start_nrt_profile.restype = ctypes.c_int64
                lib.axon_stop_nrt_profile.argtypes = [ctypes.c_char_p]
                lib.axon_stop_nrt_profile.restype = ctypes.c_int64

                @contextlib.contextmanager
                def _hook(output_dir, device_ids):
                    import jax
                    jax.devices()
                    if device_ids:
                        ids = (ctypes.c_int64 * len(device_ids))(*device_ids)
                        rc = lib.axon_start_nrt_profile(ids, len(device_ids))
                    else:
                        rc = lib.axon_start_nrt_profile(None, 0)
                    if rc != 0:
                        raise RuntimeError(f"axon_start_nrt_profile rc={rc}")
                    try:
                        yield
                    finally:
                        lib.axon_stop_nrt_profile(str(output_dir).encode())

                hook = _hook
        except OSError:
            hook = None

    mod = types.ModuleType("antenv.axon_hooks")
    mod.get_axon_ntff_profile_hook = lambda: hook
    mod.set_axon_ntff_profile_hook = lambda h: None
    sys.modules["antenv.axon_hooks"] = mod
    antenv.axon_hooks = mod


_install_ntff_hook_shim()

F32 = mybir.dt.float32
BF16 = mybir.dt.bfloat16
FP8 = mybir.dt.float8e4
AF = mybir.ActivationFunctionType
ALU = mybir.AluOpType
DR = mybir.MatmulPerfMode.DoubleRow

# problem constants
B, L, E, H, D = 2, 4096, 512, 8, 64
HID = 2048
EPS = 1e-5
WIN, DIL = 256, 2
N_CORES = 8
S = L // N_CORES          # tokens per core per batch (512)
HALO = WIN                # kv halo tokens (256)
NP = 2                    # parities
U = (S + HALO) // NP      # 384 packed tokens per parity (incl. 128 halo)
UQ = S // NP              # 256 core tokens per parity
QB = UQ // 128            # 2 query blocks per parity
KBL = U // 128            # 3 key blocks per parity
NT = NP * U               # 768 packed tokens incl halo
SP = S                    # 512 core tokens, parity-major flat
EC = E // 128              # 4
HC = HID // 128            # 16
HP = H // 2                # 4 head pairs

# dtype / scaling knobs
USE_FP8 = True
DT_W = FP8 if USE_FP8 else BF16
DT_A = FP8 if USE_FP8 else BF16
WS = 64.0 if USE_FP8 else 1.0       # host-side weight pre-scale
OS = 16.0                            # O output scale (via selbc)
NPDT = ml_dtypes.float8_e4m3 if USE_FP8 else ml_dtypes.bfloat16

# engine assignment knobs (tune from trace). NOTE: gpsimd (Pool) cannot
# access PSUM, and its ALU runs at ~0.4-0.6x — only SBUF copies belong there.
ENG = {
    "xbf": "gpsimd",     # x fp32 -> bf16 shadow (sbuf->sbuf)
    "xsq": "gpsimd",     # x^2 for LN1 stats (sbuf->sbuf)
    "xsq2": "gpsimd",    # x2^2 for LN2 stats (sbuf->sbuf)
    "musq": "gpsimd",    # mu^2 (sbuf->sbuf)
    "qcopy": "vector",   # Q psum->sbuf (+descale)
    "kcopy": "vector",   # K psum->sbuf (+descale)
    "vcopy": "scalar",   # V psum->sbuf (+descale); vector when has_v_bias
    "mask": "vector",    # probability mask multiply
    "final": "vector",   # O * rbc -> oT (psum reads)
    "mu": "scalar",      # ps_mu -> mu_bf (psum read, plain scale)
    "var": "vector",     # var = ps_sq/E - musq
    "x1": "vector",      # x1 = (x - mu) * rstd (2 ops)
    "x2": "vector",      # residual add
    "x21": "vector",     # ln2 normalize (2 ops)
    "y": "vector",       # final residual add
}


def _legalize_waits(m, max_waits=1):
    """The walrus build here accepts only one sync-wait command per lowered
    instruction; hoist extras onto same-engine NoOps placed just before."""
    for fn in m.functions:
        for blk in fn.blocks:
            new_list = []
            for ins in blk.instructions:
                si = ins.sync_info
                if si is not None and si.on_wait is not None and len(si.on_wait) > max_waits:
                    waits = list(si.on_wait)
                    extra, keep = waits[:-max_waits], waits[-max_waits:]
                    k = 0
                    while extra:
                        chunk, extra = extra[:max_waits], extra[max_waits:]
                        nop = mybir.InstNoOp(name=f"{ins.name}-wsplit{k}", ins=[], outs=[])
                        nop.engine = ins.engine
                        nop.sync_info = mybir.SyncInfo(on_wait=chunk, on_update=[])
                        new_list.append(nop)
                        k += 1
                    si.on_wait = keep
                new_list.append(ins)
            blk.instructions = new_list


def build_program(has_qk_bias: bool, has_v_bias: bool, has_out_bias: bool, has_b2: bool):
    nc = bass.Bass("TRN2", target_bir_lowering=False, debug=False)
    E2 = 2 * E
    WSI = 1.0 / WS

    def eng(site):
        return getattr(nc, ENG[site])

    # ---- DRAM I/O ----
    xp = nc.dram_tensor("xp", [B, E, NT], F32, kind="ExternalInput").ap()
    wqkv = nc.dram_tensor("wqkv", [E, 3 * E], DT_W, kind="ExternalInput").ap()
    wout = nc.dram_tensor("wout", [E, E], DT_W, kind="ExternalInput").ap()
    w1 = nc.dram_tensor("w1", [E, HID], DT_W, kind="ExternalInput").ap()
    w2 = nc.dram_tensor("w2", [HID, E], DT_W, kind="ExternalInput").ap()
    vmlp_in = nc.dram_tensor("vmlp", [HID], F32, kind="ExternalInput").ap()
    msk_in = nc.dram_tensor("msk", [128, 4 * 128], BF16, kind="ExternalInput").ap()
    ohsel_in = nc.dram_tensor("ohsel", [128, 2 * H * H], BF16, kind="ExternalInput").ap()
    selbc_in = nc.dram_tensor("selbc", [8, HP * 128], BF16, kind="ExternalInput").ap()
    if has_qk_bias:
        vqk_in = nc.dram_tensor("vqk", [2 * E], F32, kind="ExternalInput").ap()
    if has_v_bias:
        vvb_in = nc.dram_tensor("vvb", [E], F32, kind="ExternalInput").ap()
        vhalo_in = nc.dram_tensor("vhalo", [128], F32, kind="ExternalInput").ap()
    if has_out_bias:
        outb_in = nc.dram_tensor("outb", [E], F32, kind="ExternalInput").ap()
    if has_b2:
        b2_in = nc.dram_tensor("b2v", [E], F32, kind="ExternalInput").ap()
    yT = nc.dram_tensor("yT", [B, E, SP], F32, kind="ExternalOutput").ap()

    with tile.TileContext(nc) as tc:
        ctxstack = []

        def pool(name, bufs, space="SBUF"):
            p = tc.tile_pool(name=name, bufs=bufs, space=space)
            ctxstack.append(p)
            return p.__enter__()

        wpool = pool("wpool", 1)
        xpool = pool("xpool", 2)
        xbfpool = pool("xbfpool", 2)
        x1pool = pool("x1pool", 2)
        stpool = pool("stpool", 2)
        qkpool = pool("qkpool", 2)
        vpool = pool("vpool", 2)
        ptpool = pool("ptpool", 4)
        otpool = pool("otpool", 2)
        x2pool = pool("x2pool", 2)
        h2pool = pool("h2pool", 2)
        ypool = pool("ypool", 2)
        rpool = pool("rpool", 4)

        pmain = pool("pmain", 2, space="PSUM")
        psc = pool("psc", 2, space="PSUM")
        po = pool("po", 2, space="PSUM")
        pcomb = pool("pcomb", 2, space="PSUM")

        # ---- constants + tiny inputs on the gpsimd DMA queue (arrive first) ----
        msk_sb = wpool.tile([128, 4, 128], BF16)
        nc.gpsimd.dma_start(msk_sb, msk_in.rearrange("p (s q) -> p s q", s=4))
        ohsel_sb = wpool.tile([128, 2, H, H], BF16)
        nc.gpsimd.dma_start(ohsel_sb, ohsel_in.rearrange("p (k h g) -> p k h g", k=2, h=H))
        selbc_sb = wpool.tile([8, HP, 128], BF16)
        nc.gpsimd.dma_start(selbc_sb, selbc_in.rearrange("p (c q) -> p c q", c=HP))
        vmlp_sb = wpool.tile([128, HC], F32)
        nc.gpsimd.dma_start(vmlp_sb, vmlp_in.rearrange("(s p) -> p s", p=128))
        if has_qk_bias:
            vqk_sb = wpool.tile([128, 8], F32)
            nc.gpsimd.dma_start(vqk_sb, vqk_in.rearrange("(s p) -> p s", p=128))
        if has_v_bias:
            vvb_sb = wpool.tile([128, E], F32)
            nc.gpsimd.dma_start(vvb_sb, vvb_in[None, :].to_broadcast([128, E]))
            vhalo_sb = wpool.tile([128, 1], F32)
            nc.gpsimd.dma_start(vhalo_sb, vhalo_in[:, None])
        if has_out_bias:
            outb_sb = wpool.tile([128, EC], F32)
            nc.gpsimd.dma_start(outb_sb, outb_in.rearrange("(s p) -> p s", p=128))
        if has_b2:
            b2_sb = wpool.tile([128, EC], F32)
            nc.gpsimd.dma_start(b2_sb, b2_in.rearrange("(s p) -> p s", p=128))

        ones128 = wpool.tile([128, 128], BF16)
        nc.vector.memset(ones128, 1.0)
        eps_col = wpool.tile([128, 1], F32)
        nc.vector.memset(eps_col, EPS)

        # ---- big DMAs on the sync queue, ordered by first use ----
        xts = []
        for b in range(B):
            xts.append(xpool.tile([128, EC, NT], F32, tag="xt", name=f"xt{b}"))
        for c in range(EC):
            nc.sync.dma_start(xts[0][:, c], xp[0, c * 128:(c + 1) * 128, :])
        wqkv_sb = wpool.tile([128, EC, 3 * E], DT_W)
        nc.sync.dma_start(wqkv_sb, wqkv.rearrange("(c p) f -> p c f", p=128))
        for c in range(EC):
            nc.sync.dma_start(xts[1][:, c], xp[1, c * 128:(c + 1) * 128, :])
        wout_sb = wpool.tile([128, EC, E], DT_W)
        nc.sync.dma_start(wout_sb, wout.rearrange("(c p) f -> p c f", p=128))
        w1_sb = wpool.tile([128, EC, HID], DT_W)
        nc.sync.dma_start(w1_sb, w1.rearrange("(c p) f -> p c f", p=128))
        w2_sb = wpool.tile([128, HC, E], DT_W)
        nc.sync.dma_start(w2_sb, w2.rearrange("(c p) f -> p c f", p=128))

        # ================= LN stats helper =================
        def emit_stats(src_bf, sq_tiles, T):
            """src_bf: [128, EC, T] bf16; sq_tiles: list per c of [128, T] bf16.
            Returns (mu_bf, rstd_bf) [128, T] bf16 (broadcast over partitions)."""
            ntt = T // 256
            mu_bf = stpool.tile([128, T], BF16, tag="mu", name="mu")
            rstd_bf = stpool.tile([128, T], BF16, tag="rstd", name="rstd")
            for t in range(ntt):
                t0, t1 = t * 256, (t + 1) * 256
                ps = pmain.tile([128, 2, 256], F32, tag="pmain", name="ps_stat")
                for c in range(EC):
                    nc.tensor.matmul(ps[:, 0], lhsT=ones128, rhs=src_bf[:, c, t0:t1],
                                     start=(c == 0), stop=(c == EC - 1))
                for c in range(EC):
                    nc.tensor.matmul(ps[:, 1], lhsT=ones128, rhs=sq_tiles[c][:, t0:t1],
                                     start=(c == 0), stop=(c == EC - 1))
                if ENG["mu"] == "scalar":
                    nc.scalar.mul(mu_bf[:, t0:t1], ps[:, 0], 1.0 / E)
                else:
                    eng("mu").tensor_scalar(mu_bf[:, t0:t1], ps[:, 0], 1.0 / E, None, ALU.mult)
                musq = stpool.tile([128, 256], F32, tag="musq", name="musq")
                if ENG["musq"] == "scalar":
                    nc.scalar.square(musq, mu_bf[:, t0:t1])
                else:
                    eng("musq").tensor_tensor(musq, mu_bf[:, t0:t1], mu_bf[:, t0:t1], ALU.mult)
                var = stpool.tile([128, 256], F32, tag="var", name="var")
                eng("var").scalar_tensor_tensor(var, ps[:, 1], 1.0 / E, musq,
                                                ALU.mult, ALU.subtract)
                lnt = stpool.tile([128, 256], F32, tag="lnt", name="lnt")
                nc.scalar.activation(lnt, var, AF.Ln, bias=eps_col)
                nc.scalar.activation(rstd_bf[:, t0:t1], lnt, AF.Exp, scale=-0.5)
            return mu_bf, rstd_bf

        # ================= LN1 + x1, both batches =================
        x1s, xbfs = [], []
        for b in range(B):
            xt = xts[b]
            xbf = xbfpool.tile([128, EC, NT], BF16, tag="xbf", name=f"xbf{b}")
            xsqt = xbfpool.tile([128, EC, NT], BF16, tag="xsqt", name=f"xsqt{b}")
            sq_tiles = []
            for c in range(EC):
                eng("xbf").tensor_copy(xbf[:, c], xt[:, c])
                if ENG["xsq"] == "scalar":
                    nc.scalar.square(xsqt[:, c], xt[:, c])
                else:
                    eng("xsq").tensor_tensor(xsqt[:, c], xt[:, c], xt[:, c], ALU.mult)
                sq_tiles.append(xsqt[:, c])
            mu_bf, rstd_bf = emit_stats(xbf, sq_tiles, NT)
            x1 = x1pool.tile([128, EC, NT], DT_A, tag="x1", name=f"x1_{b}")
            for c in range(EC):
                t1 = x1pool.tile([128, NT], BF16, tag="x1t", name="x1t")
                eng("x1").tensor_tensor(t1, xbf[:, c], mu_bf, ALU.subtract)
                eng("x1").tensor_tensor(x1[:, c], t1, rstd_bf, ALU.mult)
            x1s.append(x1)
            xbfs.append(xbf)

        # ================= QKV =================
        def mm_acc(ps_slice, w_full, col0, rhs_fn, width):
            """Accumulate over the E contraction: w_full [128, EC, F] DT_W,
            columns [col0, col0+width); rhs_fn(c0, ncr) -> moving slice."""
            if USE_FP8:
                for j in range(EC // 2):
                    nc.tensor.matmul(ps_slice,
                                     lhsT=w_full[:, 2 * j:2 * j + 2, col0:col0 + width],
                                     rhs=rhs_fn(2 * j, 2),
                                     start=(j == 0), stop=(j == EC // 2 - 1),
                                     perf_mode=DR)
            else:
                for c in range(EC):
                    nc.tensor.matmul(ps_slice, lhsT=w_full[:, c, col0:col0 + width],
                                     rhs=rhs_fn(c, 1),
                                     start=(c == 0), stop=(c == EC - 1))

        def emit_qkv_closures(b):
            """Returns a list of closures, each emitting one QKV block."""
            x1 = x1s[b]
            x1v = x1.rearrange("p c (two u) -> p c two u", two=NP)
            qkT = qkpool.tile([128, 8, NT], BF16, tag="qkT", name=f"qkT{b}")
            qkTv = qkT.rearrange("p s (two u) -> p s two u", two=NP)
            vT = vpool.tile([128, KBL, NP, H, D], BF16, tag="vT", name=f"vT{b}")
            closures = []

            def k_block(fs, par):
                def go():
                    ps = pmain.tile([128, 512], F32, tag="pmain", name="ps_k")

                    def rhs(c0, ncr):
                        r = x1v[:, c0:c0 + ncr, par, :]
                        return r if ncr > 1 else r
                    mm_acc(ps[:, :U], wqkv_sb, E + fs * 128, rhs, 128)
                    dst = qkTv[:, 4 + fs, par, :]
                    if has_qk_bias:
                        eng("kcopy").tensor_scalar(dst, ps[:, :U], WSI,
                                                   vqk_sb[:, 4 + fs:5 + fs], ALU.mult, ALU.add)
                    else:
                        eng("kcopy").tensor_scalar(dst, ps[:, :U], WSI, None, ALU.mult)
                return go

            def q_block(fs):
                def go():
                    ps = pmain.tile([128, 512], F32, tag="pmain", name="ps_q")
                    for par in range(NP):
                        def rhs(c0, ncr, par=par):
                            return x1v[:, c0:c0 + ncr, par, 128:U]
                        mm_acc(ps[:, par * UQ:(par + 1) * UQ], wqkv_sb, fs * 128, rhs, 128)
                    dst = qkTv[:, fs, :, 128:U]
                    src = ps.rearrange("p (two u) -> p two u", two=NP)
                    if has_qk_bias:
                        eng("qcopy").tensor_scalar(dst, src, WSI,
                                                   vqk_sb[:, fs:fs + 1], ALU.mult, ALU.add)
                    else:
                        eng("qcopy").tensor_scalar(dst, src, WSI, None, ALU.mult)
                return go

            def v_block(par, kb):
                def go():
                    ps = pmain.tile([128, 512], F32, tag="pmain", name="ps_v")
                    if USE_FP8:
                        for j in range(EC // 2):
                            nc.tensor.matmul(
                                ps, lhsT=x1v[:, 2 * j:2 * j + 2, par, kb * 128:(kb + 1) * 128],
                                rhs=wqkv_sb[:, 2 * j:2 * j + 2, 2 * E:3 * E],
                                start=(j == 0), stop=(j == EC // 2 - 1), perf_mode=DR)
                    else:
                        for c in range(EC):
                            nc.tensor.matmul(
                                ps, lhsT=x1v[:, c, par, kb * 128:(kb + 1) * 128],
                                rhs=wqkv_sb[:, c, 2 * E:3 * E],
                                start=(c == 0), stop=(c == EC - 1))
                    dst = vT[:, kb, par].rearrange("p h d -> p (h d)")
                    if has_v_bias:
                        nc.vector.scalar_tensor_tensor(dst, ps, WSI, vvb_sb,
                                                       ALU.mult, ALU.add)
                        if kb == 0:
                            nc.vector.tensor_scalar(dst, dst, vhalo_sb, None, ALU.mult)
                    elif ENG["vcopy"] == "scalar":
                        nc.scalar.mul(dst, ps, WSI)
                    else:
                        eng("vcopy").tensor_scalar(dst, ps, WSI, None, ALU.mult)
                return go

            for fs in range(4):
                for par in range(NP):
                    closures.append(k_block(fs, par))
            for fs in range(4):
                closures.append(q_block(fs))
            for par in range(NP):
                for kb in range(KBL):
                    closures.append(v_block(par, kb))
            return closures, qkTv, vT

        # ================= attention =================
        def emit_att(b, qkTv, vT, oT, filler):
            oTv = oT.rearrange("p c (two u) -> p c two u", two=NP)
            fill = list(filler)
            nfill = 0

            def pop_fill(n):
                nonlocal nfill
                for _ in range(n):
                    if fill:
                        fill.pop(0)()
                        nfill += 1

            for par in range(NP):
                pcs = [pcomb.tile([128, HP, 128], F32, tag="pcomb", name=f"pc{par}_{qb}")
                       for qb in range(QB)]
                pos = [po.tile([128, HP, 128], F32, tag="po", name=f"po{par}_{qb}")
                       for qb in range(QB)]
                pend = None
                for h in range(H):
                    rb, sl = (h % 2) * 64, h // 2
                    # --- A: scores + exp + mask ---
                    ps4 = psc.tile([128, 4, 128], F32, tag="psc", name="ps_sc")
                    for qb in range(QB):
                        qs = 128 + qb * 128
                        qblk = qkTv[rb:rb + 64, sl, par, qs:qs + 128]
                        for kb in range(2):
                            ks = qb * 128 + kb * 128
                            nc.tensor.matmul(ps4[:, 2 * qb + kb],
                                             lhsT=qkTv[rb:rb + 64, 4 + sl, par, ks:ks + 128],
                                             rhs=qblk, start=True, stop=True)
                    pt = ptpool.tile([128, 4, 128], BF16, tag="pt", name="pt")
                    nc.scalar.activation(pt, ps4, AF.Exp)
                    eng("mask").tensor_tensor(pt, pt, msk_sb, ALU.mult)

                    if pend is not None:
                        pend()
                    if h % 3 == 1:
                        pop_fill(2)

                    def b_stage(h=h, rb=rb, sl=sl, pt=pt):
                        for qb in range(QB):
                            for kb in range(2):
                                kind = 0 if (qb == 0 and kb == 0) else 1
                                nc.tensor.matmul(
                                    pcs[qb][0:8, 0, :], lhsT=ohsel_sb[:, kind, h],
                                    rhs=pt[:, 2 * qb + kb],
                                    start=(h == 0 and kb == 0), stop=(h == H - 1 and kb == 1))
                            for kb in range(2):
                                nc.tensor.matmul(
                                    pos[qb][rb:rb + 64, sl, :],
                                    lhsT=vT[:, qb + kb, par, h],
                                    rhs=pt[:, 2 * qb + kb],
                                    start=(kb == 0), stop=(kb == 1))
                    pend = b_stage
                pend()
                # --- C: denominators + broadcast + final scale ---
                for qb in range(QB):
                    rden = rpool.tile([8, 128], BF16, tag="rden", name="rden")
                    with nc.allow_low_precision(reason="attn denom recip in bf16"):
                        nc.vector.reciprocal(rden, pcs[qb][0:8, 0, :])
                    for hp in range(HP):
                        nc.tensor.matmul(pcs[qb][:, hp, :], lhsT=selbc_sb[:, hp, :],
                                         rhs=rden, start=True, stop=True)
                    rbc_sb = rpool.tile([128, HP, 128], BF16, tag="rbc", name="rbc")
                    nc.scalar.copy(rbc_sb, pcs[qb])
                    eng("final").tensor_tensor(
                        oTv[:, :, par, qb * 128:(qb + 1) * 128],
                        pos[qb], rbc_sb, ALU.mult)
                pop_fill(1)
            pop_fill(len(fill))

        # ================= out-proj + LN2 + MLP =================
        def emit_proj_closures(b, oT, x2):
            xt = xts[b]
            xtv = xt.rearrange("p c (two u) -> p c two u", two=NP)
            closures = []

            def proj_block(es):
                def go():
                    ps = pmain.tile([128, 512], F32, tag="pmain", name="ps_proj")

                    def rhs(c0, ncr):
                        return oT[:, c0:c0 + ncr, :]
                    mm_acc(ps, wout_sb, es * 128, rhs, 128)
                    x2v = x2[:, es].rearrange("p (two u) -> p two u", two=NP)
                    psv = ps.rearrange("p (two u) -> p two u", two=NP)
                    scale = WSI / OS
                    eng("x2").scalar_tensor_tensor(x2v, psv, scale,
                                                   xtv[:, es, :, 128:U], ALU.mult, ALU.add)
                    if has_out_bias:
                        eng("x2").tensor_scalar(x2[:, es], x2[:, es],
                                                outb_sb[:, es:es + 1], None, ALU.add)
                return go

            for es in range(EC):
                closures.append(proj_block(es))
            return closures

        def emit_ln2_stats_closures(b, x2):
            xsq2t = xbfpool.tile([128, EC, SP], BF16, tag="xsq2t", name=f"xsq2t{b}")
            sq_tiles = [xsq2t[:, c] for c in range(EC)]
            closures = []

            def sq_block(c):
                def go():
                    eng("xsq2").tensor_tensor(xsq2t[:, c], x2[:, c], x2[:, c], ALU.mult)
                return go

            def stats_block():
                def go():
                    res.append(emit_stats(x2, sq_tiles, SP))
                return go
            res = []
            for c in range(EC):
                closures.append(sq_block(c))
            closures.append(stats_block())
            return closures, res

        def emit_x21(b, x2, mu_bf, rstd_bf):
            x21 = x2pool.tile([128, EC, SP], DT_A, tag="x21", name=f"x21_{b}")
            for c in range(EC):
                t1 = x1pool.tile([128, SP], BF16, tag="x21t", name="x21t")
                eng("x21").tensor_tensor(t1, x2[:, c], mu_bf, ALU.subtract)
                eng("x21").tensor_tensor(x21[:, c], t1, rstd_bf, ALU.mult)
            return x21

        def emit_mlp1(b, x21, h2):
            for hs in range(HC):
                ps = pmain.tile([128, 512], F32, tag="pmain", name="ps_m1")

                def rhs(c0, ncr):
                    return x21[:, c0:c0 + ncr, :]
                mm_acc(ps, w1_sb, hs * 128, rhs, 128)
                nc.scalar.activation(h2[:, hs], ps, AF.Gelu,
                                     bias=vmlp_sb[:, hs:hs + 1], scale=WSI)

        def emit_mlp2(b, h2, x2):
            for es in range(EC):
                ps = pmain.tile([128, 512], F32, tag="pmain", name="ps_m2")
                if USE_FP8:
                    for k in range(HC // 2):
                        nc.tensor.matmul(ps, lhsT=w2_sb[:, 2 * k:2 * k + 2, es * 128:(es + 1) * 128],
                                         rhs=h2[:, 2 * k:2 * k + 2, :],
                                         start=(k == 0), stop=(k == HC // 2 - 1),
                                         perf_mode=DR)
                else:
                    for hc in range(HC):
                        nc.tensor.matmul(ps, lhsT=w2_sb[:, hc, es * 128:(es + 1) * 128],
                                         rhs=h2[:, hc, :],
                                         start=(hc == 0), stop=(hc == HC - 1))
                yt = ypool.tile([128, SP], F32, tag="yt", name="yt")
                eng("y").scalar_tensor_tensor(yt, ps, WSI, x2[:, es], ALU.mult, ALU.add)
                if has_b2:
                    eng("y").tensor_scalar(yt, yt, b2_sb[:, es:es + 1], None, ALU.add)
                nc.sync.dma_start(yT[b, es * 128:(es + 1) * 128, :], yt)

        # ================= schedule =================
        qkv0, qkTv0, vT0 = emit_qkv_closures(0)
        for cl in qkv0:
            cl()
        qkv1, qkTv1, vT1 = emit_qkv_closures(1)

        oT0 = otpool.tile([128, EC, SP], DT_A, tag="oT", name="oT0")
        emit_att(0, qkTv0, vT0, oT0, qkv1)

        x2_0 = x2pool.tile([128, EC, SP], BF16, tag="x2", name="x2_0")
        proj0 = emit_proj_closures(0, oT0, x2_0)
        ln2s0, ln2res0 = emit_ln2_stats_closures(0, x2_0)

        oT1 = otpool.tile([128, EC, SP], DT_A, tag="oT", name="oT1")
        emit_att(1, qkTv1, vT1, oT1, proj0 + ln2s0)

        mu2_0, rstd2_0 = ln2res0[0]
        x21_0 = emit_x21(0, x2_0, mu2_0, rstd2_0)

        # batch 1 out-proj + LN2 (PE work overlapping batch 0's gelu stream)
        x2_1 = x2pool.tile([128, EC, SP], BF16, tag="x2", name="x2_1")
        for cl in emit_proj_closures(1, oT1, x2_1):
            cl()
        ln2s1, ln2res1 = emit_ln2_stats_closures(1, x2_1)
        for cl in ln2s1:
            cl()

        h2_0 = h2pool.tile([128, HC, SP], DT_A, tag="h2", name="h2_0")
        emit_mlp1(0, x21_0, h2_0)

        mu2_1, rstd2_1 = ln2res1[0]
        x21_1 = emit_x21(1, x2_1, mu2_1, rstd2_1)
        h2_1 = h2pool.tile([128, HC, SP], DT_A, tag="h2", name="h2_1")
        emit_mlp1(1, x21_1, h2_1)

        emit_mlp2(0, h2_0, x2_0)
        emit_mlp2(1, h2_1, x2_1)

        for p in reversed(ctxstack):
            p.__exit__(None, None, None)

    return nc


_cached = {}


def _get_program(key):
    if key not in _cached:
        nc = build_program(*key)
        _legalize_waits(nc.m)
        _cached[key] = nc
    return _cached[key]


def _prepare_core_inputs(inputs):
    """Host-side folding + parity packing + sharding."""
    x = np.asarray(inputs["x"], np.float32)
    ln1_g = np.asarray(inputs["ln1_g"], np.float32)
    ln1_b = np.asarray(inputs["ln1_b"], np.float32)
    qkv_w = np.asarray(inputs["qkv_w"], np.float32)
    qkv_b = np.asarray(inputs["qkv_b"], np.float32)
    out_w = np.asarray(inputs["out_w"], np.float32)
    out_b = np.asarray(inputs["out_b"], np.float32)
    ln2_g = np.asarray(inputs["ln2_g"], np.float32)
    ln2_b = np.asarray(inputs["ln2_b"], np.float32)
    w1 = np.asarray(inputs["w1"], np.float32)
    b1 = np.asarray(inputs["b1"], np.float32)
    w2 = np.asarray(inputs["w2"], np.float32)
    b2 = np.asarray(inputs["b2"], np.float32)

    qscale = 1.0 / np.sqrt(D)
    wqkv_eff = ln1_g[:, None] * qkv_w
    vqkv = ln1_b @ qkv_w + qkv_b
    wqkv_eff[:, :E] *= qscale
    vqkv = vqkv.copy()
    vqkv[:E] *= qscale
    w1_eff = ln2_g[:, None] * w1
    vmlp = ln2_b @ w1 + b1

    has_qk_bias = bool(np.any(vqkv[: 2 * E] != 0.0))
    has_v_bias = bool(np.any(vqkv[2 * E:] != 0.0))
    has_out_bias = bool(np.any(out_b != 0.0))
    has_b2 = bool(np.any(b2 != 0.0))
    key = (has_qk_bias, has_v_bias, has_out_bias, has_b2)

    wqkv_c = (wqkv_eff * WS).astype(NPDT)
    wout_c = (out_w * WS).astype(NPDT)
    w1_c = (w1_eff * WS).astype(NPDT)
    w2_c = (w2 * WS).astype(NPDT)

    # masks: slot 0/2 = prev-block (keep q<=k), slot 1/3 = diag (keep k<=q)
    ki = np.arange(128)[:, None]
    qi = np.arange(128)[None, :]
    mprev = (qi <= ki).astype(np.float32)
    mdiag = (ki <= qi).astype(np.float32)
    msk = np.stack([mprev, mdiag, mprev, mdiag], axis=1)     # [128, 4, 128]
    msk = msk.reshape(128, 4 * 128).astype(ml_dtypes.bfloat16)

    # selbc: [8, HP, 128]: OS where h == 2*hp + (p>=64)
    selbc = np.zeros((8, HP, 128), np.float32)
    for hp in range(HP):
        selbc[2 * hp, hp, :64] = OS
        selbc[2 * hp + 1, hp, 64:] = OS
    selbc = selbc.reshape(8, HP * 128).astype(ml_dtypes.bfloat16)

    # x: transpose + halo + parity pack: [B, E, NP, U] -> flat [B, E, NT]
    xT_full = np.ascontiguousarray(x.transpose(0, 2, 1))  # [B, E, L]
    in_maps = []
    for core in range(N_CORES):
        s = core * S
        slab = np.zeros((B, E, S + HALO), np.float32)
        lo = s - HALO
        src_lo = max(lo, 0)
        slab[:, :, src_lo - lo:] = xT_full[:, :, src_lo:s + S]
        xpk = slab.reshape(B, E, U, NP).transpose(0, 1, 3, 2)  # [B, E, NP, U]
        xpk = np.ascontiguousarray(xpk).reshape(B, E, NT)

        # ohsel: [128, 2, H, H]: kind 0 = halo-valid one-hot, kind 1 = ones one-hot
        ohsel = np.zeros((128, 2, H, H), np.float32)
        halo_valid = 0.0 if core == 0 else 1.0
        for h in range(H):
            ohsel[:, 0, h, h] = halo_valid
            ohsel[:, 1, h, h] = 1.0
        ohsel = ohsel.reshape(128, 2 * H * H).astype(ml_dtypes.bfloat16)

        im = {
            "xp": xpk,
            "wqkv": wqkv_c,
            "wout": wout_c,
            "w1": w1_c,
            "w2": w2_c,
            "vmlp": vmlp.astype(np.float32),
            "msk": msk,
            "ohsel": ohsel,
            "selbc": selbc,
        }
        if has_qk_bias:
            im["vqk"] = vqkv[: 2 * E].astype(np.float32)
        if has_v_bias:
            im["vvb"] = vqkv[2 * E:].astype(np.float32)
            im["vhalo"] = np.full(128, halo_valid, np.float32)
        if has_out_bias:
            im["outb"] = out_b.astype(np.float32)
        if has_b2:
            im["b2v"] = b2.astype(np.float32)
        in_maps.append(im)
    return key, in_maps


_last_results = None


def kernel(**inputs) -> np.ndarray:
    global _last_results
    key, in_maps = _prepare_core_inputs(inputs)
    nc = _get_program(key)
    res = run_bass_kernel_spmd(nc, in_maps, core_ids=list(range(N_CORES)))
    _last_results = res
    out = np.empty((B, L, E), np.float32)
    for core in range(N_CORES):
        yp = res.results[core]["yT"]          # [B, E, SP] parity-packed
        yu = yp.reshape(B, E, NP, UQ).transpose(0, 1, 3, 2).reshape(B, E, S)
        out[:, core * S:(core + 1) * S, :] = yu.transpose(0, 2, 1)
    return out


# revision 19
# speedup vs baseline: 1.4872x; 1.1384x over previous
"""Trainium2 Bass kernel for nn_DilatedAttention (B=2, L=4096, E=512, H=8, D=64,
dilation=2, window=256, causal, pre-norm transformer block with MLP).

Strategy (v2)
-------------
* 8 cores, sequence-parallel: core c owns tokens [512c, 512c+512) of both
  batches, with a 256-token K/V halo (zero-padded on core 0).
* Dilation-2 + causal + window couples only equal-parity tokens. The HOST
  packs tokens parity-major (all even tokens, then all odd) so every on-chip
  access is stride-1; the host un-packs the output. In parity space the mask
  is a causal sliding window of 128: for each 128-query block only the
  previous and the diagonal 128-key blocks matter, masked by two constant
  triangular 0/1 matrices (multiplied into the probabilities on DVE).
* Attention produces O directly feature-major: the O matmul uses V as the
  128-stationary ([128 keys, 64]) and the probabilities as moving, with PE
  column-tiling packing two heads per PSUM tile. Softmax denominators come
  from per-head one-hot rank-8 matmuls accumulated into one [8, 128] PSUM
  row-block, reciprocal on DVE, broadcast back with a constant selector
  matmul. No transposes anywhere.
* LayerNorm stats via ones[128,128] matmuls (broadcast across partitions for
  free); rstd = Exp(-0.5*Ln(var+eps)) on the Act engine so the whole kernel
  needs only the {ln,exp} and {gelu} activation tables (square/copy/identity
  are in every table set).
* All big GEMMs (QKV, out-proj, MLP) run in fp8e4 with DoubleRow perf mode
  (2 contraction tiles per pass). Weights are pre-scaled by 64 on the host
  (fp8 subnormal avoidance) and descaled in the PSUM->SBUF copies.
  Attention score/O matmuls stay bf16.
"""

import os
import sys
import types
import numpy as np
import ml_dtypes

import concourse.bass as bass
import concourse.mybir as mybir
import concourse.tile as tile
from concourse.bass_utils import run_bass_kernel_spmd


def _install_ntff_hook_shim():
    """This image's antenv lacks axon_hooks; bass_utils imports it when
    BASS_TRACE is set.  Provide the ctypes-based NTFF hook (or a None hook)
    so tracing works — and never crashes — in any environment."""
    try:
        import antenv
    except ImportError:
        return
    try:
        from antenv.axon_hooks import get_axon_ntff_profile_hook  # noqa: F401
        return  # real module present
    except ImportError:
        pass
    import ctypes
    import contextlib

    hook = None
    so_path = "/opt/axon/libaxon_pjrt.so"
    if os.path.exists(so_path):
        try:
            lib = ctypes.CDLL(so_path)
            if hasattr(lib, "axon_start_nrt_profile"):
                lib.axon_start_nrt_profile.argtypes = [
                    ctypes.POINTER(ctypes.c_int64), ctypes.c_size_t]
                lib.axon_start_nrt_profile.restype = ctypes.c_int64
                lib.axon_stop_nrt_profile.argtypes = [ctypes.c_char_p]
                lib.axon_stop_nrt_profile.restype = ctypes.c_int64

                @contextlib.contextmanager
                def _hook(output_dir, device_ids):
                    import jax
                    jax.devices()
                    if device_ids:
                        ids = (ctypes.c_int64 * len(device_ids))(*device_ids)
                        rc = lib.axon_start_nrt_profile(ids, len(device_ids))
                    else:
                        rc = lib.axon_start_nrt_profile(None, 0)
                    if rc != 0:
                        raise RuntimeError(f"axon_start_nrt_profile rc={rc}")
                    try:
                        yield
                    finally:
                        lib.axon_stop_nrt_profile(str(output_dir).encode())

                hook = _hook
        except OSError:
            hook = None

    mod = types.ModuleType("antenv.axon_hooks")
    mod.get_axon_ntff_profile_hook = lambda: hook
    mod.set_axon_ntff_profile_hook = lambda h: None
    sys.modules["antenv.axon_hooks"] = mod
    antenv.axon_hooks = mod


_install_ntff_hook_shim()

F32 = mybir.dt.float32
BF16 = mybir.dt.bfloat16
FP8 = mybir.dt.float8e4
AF = mybir.ActivationFunctionType
ALU = mybir.AluOpType
DR = mybir.MatmulPerfMode.DoubleRow

# problem constants
B, L, E, H, D = 2, 4096, 512, 8, 64
HID = 2048
EPS = 1e-5
WIN, DIL = 256, 2
N_CORES = 8
S = L // N_CORES          # tokens per core per batch (512)
HALO = WIN                # kv halo tokens (256)
NP = 2                    # parities
U = (S + HALO) // NP      # 384 packed tokens per parity (incl. 128 halo)
UQ = S // NP              # 256 core tokens per parity
QB = UQ // 128            # 2 query blocks per parity
KBL = U // 128            # 3 key blocks per parity
NT = NP * U               # 768 packed tokens incl halo
SP = S                    # 512 core tokens, parity-major flat
EC = E // 128              # 4
HC = HID // 128            # 16
HP = H // 2                # 4 head pairs

# dtype / scaling knobs
USE_FP8 = True
DT_W = FP8 if USE_FP8 else BF16
DT_A = FP8 if USE_FP8 else BF16
WS = 64.0 if USE_FP8 else 1.0       # host-side weight pre-scale
OS = 16.0                            # O output scale (via selbc)
NPDT = ml_dtypes.float8_e4m3 if USE_FP8 else ml_dtypes.bfloat16

# engine assignment knobs (tune from trace). NOTE: gpsimd (Pool) cannot
# access PSUM, and its ALU runs at ~0.4-0.6x — only SBUF work belongs there.
ENG = {
    "xbf": "scalar",     # x fp32 -> bf16 shadow (table-free Act copy)
    "xsq": "vector",     # x^2 for LN1 stats (sbuf->sbuf)
    "xsq2": "gpsimd",    # x2^2 for LN2 stats (sbuf->sbuf)
    "musq": "gpsimd",    # mu^2 (sbuf->sbuf)
    "qcopy": "vector",   # Q psum->sbuf (+descale)
    "kcopy": "vector",   # K psum->sbuf (+descale)
    "vcopy": "vector",   # V psum->sbuf (+descale)
    "final": "vector",   # O * rbc -> oT (psum reads)
    "mu": "scalar",      # ps_mu -> mu_bf (psum read, plain scale)
    "var": "vector",     # var = ps_sq/E - musq
    "x1": "vector",      # x1 sub op
    "x1m": ("vector", "vector", "gpsimd", "gpsimd"),   # x1 mult per chunk
    "x2": "vector",      # residual add
    "x21": "vector",     # ln2 normalize sub
    "x21m": ("vector", "vector", "gpsimd", "gpsimd"),  # x21 mult per chunk
    "y": "vector",       # final residual add
}


def _legalize_waits(m, max_waits=1):
    """The walrus build here accepts only one sync-wait command per lowered
    instruction; hoist extras onto same-engine NoOps placed just before."""
    for fn in m.functions:
        for blk in fn.blocks:
            new_list = []
            for ins in blk.instructions:
                si = ins.sync_info
                if si is not None and si.on_wait is not None and len(si.on_wait) > max_waits:
                    waits = list(si.on_wait)
                    extra, keep = waits[:-max_waits], waits[-max_waits:]
                    k = 0
                    while extra:
                        chunk, extra = extra[:max_waits], extra[max_waits:]
                        nop = mybir.InstNoOp(name=f"{ins.name}-wsplit{k}", ins=[], outs=[])
                        nop.engine = ins.engine
                        nop.sync_info = mybir.SyncInfo(on_wait=chunk, on_update=[])
                        new_list.append(nop)
                        k += 1
                    si.on_wait = keep
                new_list.append(ins)
            blk.instructions = new_list


def build_program(has_qk_bias: bool, has_v_bias: bool, has_out_bias: bool, has_b2: bool):
    nc = bass.Bass("TRN2", target_bir_lowering=False, debug=False)
    E2 = 2 * E
    WSI = 1.0 / WS

    def eng(site):
        return getattr(nc, ENG[site])

    # ---- DRAM I/O ----
    xp = nc.dram_tensor("xp", [B, E, NT], F32, kind="ExternalInput").ap()
    wqkv = nc.dram_tensor("wqkv", [E, 3 * E], DT_W, kind="ExternalInput").ap()
    wout = nc.dram_tensor("wout", [E, E], DT_W, kind="ExternalInput").ap()
    w1 = nc.dram_tensor("w1", [E, HID], DT_W, kind="ExternalInput").ap()
    w2 = nc.dram_tensor("w2", [HID, E], DT_W, kind="ExternalInput").ap()
    vmlp_in = nc.dram_tensor("vmlp", [HID], F32, kind="ExternalInput").ap()
    msk_in = nc.dram_tensor("msk", [128, 4 * 128], BF16, kind="ExternalInput").ap()
    ohsel_in = nc.dram_tensor("ohsel", [128, 2 * H * H], BF16, kind="ExternalInput").ap()
    selbc_in = nc.dram_tensor("selbc", [8, HP * 128], BF16, kind="ExternalInput").ap()
    if has_qk_bias:
        vqk_in = nc.dram_tensor("vqk", [2 * E], F32, kind="ExternalInput").ap()
    if has_v_bias:
        vvb_in = nc.dram_tensor("vvb", [E], F32, kind="ExternalInput").ap()
        vhalo_in = nc.dram_tensor("vhalo", [128], F32, kind="ExternalInput").ap()
    if has_out_bias:
        outb_in = nc.dram_tensor("outb", [E], F32, kind="ExternalInput").ap()
    if has_b2:
        b2_in = nc.dram_tensor("b2v", [E], F32, kind="ExternalInput").ap()
    yT = nc.dram_tensor("yT", [B, E, SP], F32, kind="ExternalOutput").ap()

    with tile.TileContext(nc) as tc:
        ctxstack = []

        def pool(name, bufs, space="SBUF"):
            p = tc.tile_pool(name=name, bufs=bufs, space=space)
            ctxstack.append(p)
            return p.__enter__()

        wpool = pool("wpool", 1)
        xpool = pool("xpool", 2)
        xbfpool = pool("xbfpool", 2)
        x1pool = pool("x1pool", 2)
        stpool = pool("stpool", 2)
        qkpool = pool("qkpool", 2)
        vpool = pool("vpool", 2)
        ptpool = pool("ptpool", 4)
        otpool = pool("otpool", 2)
        x2pool = pool("x2pool", 2)
        h2pool = pool("h2pool", 2)
        ypool = pool("ypool", 2)
        rpool = pool("rpool", 4)

        pmain = pool("pmain", 2, space="PSUM")
        psc = pool("psc", 2, space="PSUM")
        po = pool("po", 2, space="PSUM")
        pcomb = pool("pcomb", 2, space="PSUM")

        # ---- constants + tiny inputs on the gpsimd DMA queue (arrive first) ----
        msk_sb = wpool.tile([128, 4, 128], BF16)
        nc.gpsimd.dma_start(msk_sb, msk_in.rearrange("p (s q) -> p s q", s=4))
        ohsel_sb = wpool.tile([128, 2, H, H], BF16)
        nc.gpsimd.dma_start(ohsel_sb, ohsel_in.rearrange("p (k h g) -> p k h g", k=2, h=H))
        selbc_sb = wpool.tile([8, HP, 128], BF16)
        nc.gpsimd.dma_start(selbc_sb, selbc_in.rearrange("p (c q) -> p c q", c=HP))
        vmlp_sb = wpool.tile([128, HC], F32)
        nc.gpsimd.dma_start(vmlp_sb, vmlp_in.rearrange("(s p) -> p s", p=128))
        if has_qk_bias:
            vqk_sb = wpool.tile([128, 8], F32)
            nc.gpsimd.dma_start(vqk_sb, vqk_in.rearrange("(s p) -> p s", p=128))
        if has_v_bias:
            vvb_sb = wpool.tile([128, E], F32)
            nc.gpsimd.dma_start(vvb_sb, vvb_in[None, :].to_broadcast([128, E]))
            vhalo_sb = wpool.tile([128, 1], F32)
            nc.gpsimd.dma_start(vhalo_sb, vhalo_in[:, None])
        if has_out_bias:
            outb_sb = wpool.tile([128, EC], F32)
            nc.gpsimd.dma_start(outb_sb, outb_in.rearrange("(s p) -> p s", p=128))
        if has_b2:
            b2_sb = wpool.tile([128, EC], F32)
            nc.gpsimd.dma_start(b2_sb, b2_in.rearrange("(s p) -> p s", p=128))

        ones128 = wpool.tile([128, 128], BF16)
        nc.vector.memset(ones128, 1.0)
        eps_col = wpool.tile([128, 1], F32)
        nc.vector.memset(eps_col, EPS)

        # ---- big DMAs on the sync queue, ordered by first use ----
        xts = []
        for b in range(B):
            xts.append(xpool.tile([128, EC, NT], F32, tag="xt", name=f"xt{b}"))
        for c in range(EC):
            nc.sync.dma_start(xts[0][:, c], xp[0, c * 128:(c + 1) * 128, :])
        wqkv_sb = wpool.tile([128, EC, 3 * E], DT_W)
        nc.sync.dma_start(wqkv_sb, wqkv.rearrange("(c p) f -> p c f", p=128))
        for c in range(EC):
            nc.sync.dma_start(xts[1][:, c], xp[1, c * 128:(c + 1) * 128, :])
        wout_sb = wpool.tile([128, EC, E], DT_W)
        nc.sync.dma_start(wout_sb, wout.rearrange("(c p) f -> p c f", p=128))
        w1_sb = wpool.tile([128, EC, HID], DT_W)
        nc.sync.dma_start(w1_sb, w1.rearrange("(c p) f -> p c f", p=128))
        w2_sb = wpool.tile([128, HC, E], DT_W)
        nc.sync.dma_start(w2_sb, w2.rearrange("(c p) f -> p c f", p=128))

        # ================= LN stats helper =================
        def emit_stats(xstat, T):
            """xstat: [128, EC, 2, T] bf16 with slot 0 = x, slot 1 = x^2.
            Returns (mu_bf, rstd_bf) [128, T] bf16 (broadcast over partitions)."""
            ntt = T // 256
            mu_bf = stpool.tile([128, T], BF16, tag="mu", name="mu")
            rstd_bf = stpool.tile([128, T], BF16, tag="rstd", name="rstd")
            for t in range(ntt):
                t0, t1 = t * 256, (t + 1) * 256
                ps = pmain.tile([128, 2, 256], F32, tag="pmain", name="ps_stat")
                for c in range(EC):
                    nc.tensor.matmul(ps, lhsT=ones128, rhs=xstat[:, c, :, t0:t1],
                                     start=(c == 0), stop=(c == EC - 1))
                if ENG["mu"] == "scalar":
                    nc.scalar.mul(mu_bf[:, t0:t1], ps[:, 0], 1.0 / E)
                else:
                    eng("mu").tensor_scalar(mu_bf[:, t0:t1], ps[:, 0], 1.0 / E, None, ALU.mult)
                musq = stpool.tile([128, 256], F32, tag="musq", name="musq")
                if ENG["musq"] == "scalar":
                    nc.scalar.square(musq, mu_bf[:, t0:t1])
                else:
                    eng("musq").tensor_tensor(musq, mu_bf[:, t0:t1], mu_bf[:, t0:t1], ALU.mult)
                var = stpool.tile([128, 256], F32, tag="var", name="var")
                eng("var").scalar_tensor_tensor(var, ps[:, 1], 1.0 / E, musq,
                                                ALU.mult, ALU.subtract)
                lnt = stpool.tile([128, 256], F32, tag="lnt", name="lnt")
                nc.scalar.activation(lnt, var, AF.Ln, bias=eps_col)
                nc.scalar.activation(rstd_bf[:, t0:t1], lnt, AF.Exp, scale=-0.5)
            return mu_bf, rstd_bf

        # ================= LN1 + x1, both batches =================
        x1s = []
        for b in range(B):
            xt = xts[b]
            xstat = xbfpool.tile([128, EC, 2, NT], BF16, tag="xstat", name=f"xstat{b}")
            for c in range(EC):
                if ENG["xbf"] == "scalar":
                    nc.scalar.copy(xstat[:, c, 0], xt[:, c])
                else:
                    eng("xbf").tensor_copy(xstat[:, c, 0], xt[:, c])
                if ENG["xsq"] == "scalar":
                    nc.scalar.square(xstat[:, c, 1], xt[:, c])
                else:
                    eng("xsq").tensor_tensor(xstat[:, c, 1], xt[:, c], xt[:, c], ALU.mult)
            mu_bf, rstd_bf = emit_stats(xstat, NT)
            x1 = x1pool.tile([128, EC, NT], DT_A, tag="x1", name=f"x1_{b}")
            for c in range(EC):
                t1 = x1pool.tile([128, NT], BF16, tag="x1t", name="x1t")
                eng("x1").tensor_tensor(t1, xstat[:, c, 0], mu_bf, ALU.subtract)
                getattr(nc, ENG["x1m"][c]).tensor_tensor(x1[:, c], t1, rstd_bf, ALU.mult)
            x1s.append(x1)

        # ================= QKV =================
        def mm_acc(ps_slice, w_full, col0, rhs_fn, width):
            """Accumulate over the E contraction: w_full [128, EC, F] DT_W,
            columns [col0, col0+width); rhs_fn(c0, ncr) -> moving slice."""
            if USE_FP8:
                for j in range(EC // 2):
                    nc.tensor.matmul(ps_slice,
                                     lhsT=w_full[:, 2 * j:2 * j + 2, col0:col0 + width],
                                     rhs=rhs_fn(2 * j, 2),
                                     start=(j == 0), stop=(j == EC // 2 - 1),
                                     perf_mode=DR)
            else:
                for c in range(EC):
                    nc.tensor.matmul(ps_slice, lhsT=w_full[:, c, col0:col0 + width],
                                     rhs=rhs_fn(c, 1),
                                     start=(c == 0), stop=(c == EC - 1))

        def emit_qkv_closures(b):
            """Returns a list of closures, each emitting one QKV block."""
            x1 = x1s[b]
            x1v = x1.rearrange("p c (two u) -> p c two u", two=NP)
            qkT = qkpool.tile([128, 8, NT], BF16, tag="qkT", name=f"qkT{b}")
            qkTv = qkT.rearrange("p s (two u) -> p s two u", two=NP)
            vT = vpool.tile([128, KBL, NP, H, D], BF16, tag="vT", name=f"vT{b}")
            closures = []

            def k_block(fs, par):
                def go():
                    ps = pmain.tile([128, 512], F32, tag="pmain", name="ps_k")

                    def rhs(c0, ncr):
                        r = x1v[:, c0:c0 + ncr, par, :]
                        return r if ncr > 1 else r
                    mm_acc(ps[:, :U], wqkv_sb, E + fs * 128, rhs, 128)
                    dst = qkTv[:, 4 + fs, par, :]
                    if has_qk_bias:
                        eng("kcopy").tensor_scalar(dst, ps[:, :U], WSI,
                                                   vqk_sb[:, 4 + fs:5 + fs], ALU.mult, ALU.add)
                    else:
                        eng("kcopy").tensor_scalar(dst, ps[:, :U], WSI, None, ALU.mult)
                return go

            def q_block(fs):
                def go():
                    ps = pmain.tile([128, 512], F32, tag="pmain", name="ps_q")

                    def rhs(c0, ncr):
                        return x1v[:, c0:c0 + ncr, :, 128:U]
                    mm_acc(ps, wqkv_sb, fs * 128, rhs, 128)
                    dst = qkTv[:, fs, :, 128:U]
                    src = ps.rearrange("p (two u) -> p two u", two=NP)
                    if has_qk_bias:
                        eng("qcopy").tensor_scalar(dst, src, WSI,
                                                   vqk_sb[:, fs:fs + 1], ALU.mult, ALU.add)
                    else:
                        eng("qcopy").tensor_scalar(dst, src, WSI, None, ALU.mult)
                return go

            def v_block(par, kb):
                def go():
                    ps = pmain.tile([128, 512], F32, tag="pmain", name="ps_v")
                    if USE_FP8:
                        for j in range(EC // 2):
                            nc.tensor.matmul(
                                ps, lhsT=x1v[:, 2 * j:2 * j + 2, par, kb * 128:(kb + 1) * 128],
                                rhs=wqkv_sb[:, 2 * j:2 * j + 2, 2 * E:3 * E],
                                start=(j == 0), stop=(j == EC // 2 - 1), perf_mode=DR)
                    else:
                        for c in range(EC):
                            nc.tensor.matmul(
                                ps, lhsT=x1v[:, c, par, kb * 128:(kb + 1) * 128],
                                rhs=wqkv_sb[:, c, 2 * E:3 * E],
                                start=(c == 0), stop=(c == EC - 1))
                    dst = vT[:, kb, par].rearrange("p h d -> p (h d)")
                    if has_v_bias:
                        nc.vector.scalar_tensor_tensor(dst, ps, WSI, vvb_sb,
                                                       ALU.mult, ALU.add)
                        if kb == 0:
                            nc.vector.tensor_scalar(dst, dst, vhalo_sb, None, ALU.mult)
                    elif ENG["vcopy"] == "scalar":
                        nc.scalar.mul(dst, ps, WSI)
                    else:
                        eng("vcopy").tensor_scalar(dst, ps, WSI, None, ALU.mult)
                return go

            for fs in range(4):
                for par in range(NP):
                    closures.append(k_block(fs, par))
            for fs in range(4):
                closures.append(q_block(fs))
            for par in range(NP):
                for kb in range(KBL):
                    closures.append(v_block(par, kb))
            return closures, qkTv, vT

        # ================= attention =================
        def emit_att(b, qkTv, vT, oT, filler):
            oTv = oT.rearrange("p c (two u) -> p c two u", two=NP)
            fill = list(filler)
            nfill = 0

            def pop_fill(n):
                nonlocal nfill
                for _ in range(n):
                    if fill:
                        fill.pop(0)()
                        nfill += 1

            for par in range(NP):
                pcs = [pcomb.tile([128, HP, 128], F32, tag="pcomb", name=f"pc{par}_{qb}")
                       for qb in range(QB)]
                # O accum: two tiles per par, [128, hp-pair, qb, 128]
                pos = [po.tile([128, 2, QB, 128], F32, tag="po", name=f"po{par}_{g}")
                       for g in range(2)]
                pend = None
                for h in range(H):
                    rb, sl = (h % 2) * 64, h // 2
                    # --- A: scores (3 mms) + exp + mask (pool selects) ---
                    ps4 = psc.tile([128, 4, 128], F32, tag="psc", name="ps_sc")
                    kv = qkTv[rb:rb + 64, 4 + sl, par, :]
                    qv = qkTv[rb:rb + 64, sl, par, :]
                    nc.tensor.matmul(ps4[:, 0], lhsT=kv[:, 0:128],
                                     rhs=qv[:, 128:256], start=True, stop=True)
                    nc.tensor.matmul(ps4[:, 1:3], lhsT=kv[:, 128:256],
                                     rhs=qv[:, 128:U], start=True, stop=True)
                    nc.tensor.matmul(ps4[:, 3], lhsT=kv[:, 256:U],
                                     rhs=qv[:, 256:U], start=True, stop=True)
                    pt = ptpool.tile([128, 4, 128], BF16, tag="pt", name="pt")
                    nc.scalar.activation(pt, ps4, AF.Exp)
                    ptr = pt.rearrange("p (a k) q -> p k a q", k=2)
                    # prev-block slots (0, 2): keep q <= k
                    nc.gpsimd.affine_select(
                        out=ptr[:, 0], in_=ptr[:, 0], compare_op=ALU.is_ge,
                        fill=0.0, base=0, channel_multiplier=1,
                        pattern=[[0, 2], [-1, 128]])
                    # diag slots (1, 3): keep k <= q
                    nc.gpsimd.affine_select(
                        out=ptr[:, 1], in_=ptr[:, 1], compare_op=ALU.is_ge,
                        fill=0.0, base=0, channel_multiplier=-1,
                        pattern=[[0, 2], [1, 128]])

                    if pend is not None:
                        pend()
                    if h % 3 == 1:
                        pop_fill(2)

                    def b_stage(h=h, rb=rb, sl=sl, pt=pt):
                        for qb in range(QB):
                            for kb in range(2):
                                kind = 0 if (qb == 0 and kb == 0) else 1
                                nc.tensor.matmul(
                                    pcs[qb][0:8, 0, :], lhsT=ohsel_sb[:, kind, h],
                                    rhs=pt[:, 2 * qb + kb],
                                    start=(h == 0 and kb == 0), stop=(h == H - 1 and kb == 1))
                        g, hh = sl // 2, sl % 2
                        # k0 feeds qb0-diag and qb1-prev in one 256-col pass
                        nc.tensor.matmul(
                            pos[g][rb:rb + 64, hh, :, :], lhsT=vT[:, 1, par, h],
                            rhs=pt[:, 1:3], start=True, stop=False,
                            skip_group_check=True)
                        nc.tensor.matmul(
                            pos[g][rb:rb + 64, hh, 0, :], lhsT=vT[:, 0, par, h],
                            rhs=pt[:, 0], start=False, stop=True,
                            skip_group_check=True)
                        nc.tensor.matmul(
                            pos[g][rb:rb + 64, hh, 1, :], lhsT=vT[:, 2, par, h],
                            rhs=pt[:, 3], start=False, stop=True,
                            skip_group_check=True)
                    pend = b_stage
                pend()
                # --- C: denominators + broadcast + final scale ---
                for qb in range(QB):
                    rden = rpool.tile([8, 128], BF16, tag="rden", name="rden")
                    with nc.allow_low_precision(reason="attn denom recip in bf16"):
                        nc.vector.reciprocal(rden, pcs[qb][0:8, 0, :])
                    for hp in range(HP):
                        nc.tensor.matmul(pcs[qb][:, hp, :], lhsT=selbc_sb[:, hp, :],
                                         rhs=rden, start=True, stop=True)
                    rbc_sb = rpool.tile([128, HP, 128], BF16, tag="rbc", name="rbc")
                    nc.scalar.copy(rbc_sb, pcs[qb])
                    for g in range(2):
                        eng("final").tensor_tensor(
                            oTv[:, 2 * g:2 * g + 2, par, qb * 128:(qb + 1) * 128],
                            pos[g][:, :, qb, :], rbc_sb[:, 2 * g:2 * g + 2, :], ALU.mult)
                pop_fill(1)
            pop_fill(len(fill))

        # ================= out-proj + LN2 + MLP =================
        def emit_proj_closures(b, oT, x2):
            xt = xts[b]
            xtv = xt.rearrange("p c (two u) -> p c two u", two=NP)
            closures = []

            def proj_block(es):
                def go():
                    ps = pmain.tile([128, 512], F32, tag="pmain", name="ps_proj")

                    def rhs(c0, ncr):
                        return oT[:, c0:c0 + ncr, :]
                    mm_acc(ps, wout_sb, es * 128, rhs, 128)
                    x2v = x2[:, es, 0].rearrange("p (two u) -> p two u", two=NP)
                    psv = ps.rearrange("p (two u) -> p two u", two=NP)
                    scale = WSI / OS
                    eng("x2").scalar_tensor_tensor(x2v, psv, scale,
                                                   xtv[:, es, :, 128:U], ALU.mult, ALU.add)
                    if has_out_bias:
                        eng("x2").tensor_scalar(x2[:, es, 0], x2[:, es, 0],
                                                outb_sb[:, es:es + 1], None, ALU.add)
                return go

            for es in range(EC):
                closures.append(proj_block(es))
            return closures

        def emit_ln2_stats_closures(b, x2):
            closures = []

            def sq_block(c):
                def go():
                    eng("xsq2").tensor_tensor(x2[:, c, 1], x2[:, c, 0], x2[:, c, 0],
                                              ALU.mult)
                return go

            def stats_block():
                def go():
                    res.append(emit_stats(x2, SP))
                return go
            res = []
            for c in range(EC):
                closures.append(sq_block(c))
            closures.append(stats_block())
            return closures, res

        def emit_x21(b, x2, mu_bf, rstd_bf):
            x21 = x2pool.tile([128, EC, SP], DT_A, tag="x21", name=f"x21_{b}")
            for c in range(EC):
                t1 = x1pool.tile([128, SP], BF16, tag="x21t", name="x21t")
                eng("x21").tensor_tensor(t1, x2[:, c, 0], mu_bf, ALU.subtract)
                getattr(nc, ENG["x21m"][c]).tensor_tensor(x21[:, c], t1, rstd_bf, ALU.mult)
            return x21

        def emit_mlp1(b, x21, h2):
            for hs in range(HC):
                ps = pmain.tile([128, 512], F32, tag="pmain", name="ps_m1")

                def rhs(c0, ncr):
                    return x21[:, c0:c0 + ncr, :]
                mm_acc(ps, w1_sb, hs * 128, rhs, 128)
                nc.scalar.activation(h2[:, hs], ps, AF.Gelu,
                                     bias=vmlp_sb[:, hs:hs + 1], scale=WSI)

        def emit_mlp2(b, h2, x2):
            for es in range(EC):
                ps = pmain.tile([128, 512], F32, tag="pmain", name="ps_m2")
                if USE_FP8:
                    for k in range(HC // 2):
                        nc.tensor.matmul(ps, lhsT=w2_sb[:, 2 * k:2 * k + 2, es * 128:(es + 1) * 128],
                                         rhs=h2[:, 2 * k:2 * k + 2, :],
                                         start=(k == 0), stop=(k == HC // 2 - 1),
                                         perf_mode=DR)
                else:
                    for hc in range(HC):
                        nc.tensor.matmul(ps, lhsT=w2_sb[:, hc, es * 128:(es + 1) * 128],
                                         rhs=h2[:, hc, :],
                                         start=(hc == 0), stop=(hc == HC - 1))
                yt = ypool.tile([128, SP], F32, tag="yt", name="yt")
                eng("y").scalar_tensor_tensor(yt, ps, WSI, x2[:, es, 0], ALU.mult, ALU.add)
                if has_b2:
                    eng("y").tensor_scalar(yt, yt, b2_sb[:, es:es + 1], None, ALU.add)
                nc.sync.dma_start(yT[b, es * 128:(es + 1) * 128, :], yt)

        # ================= schedule =================
        qkv0, qkTv0, vT0 = emit_qkv_closures(0)
        for cl in qkv0:
            cl()
        qkv1, qkTv1, vT1 = emit_qkv_closures(1)

        oT0 = otpool.tile([128, EC, SP], DT_A, tag="oT", name="oT0")
        emit_att(0, qkTv0, vT0, oT0, qkv1)

        x2_0 = x2pool.tile([128, EC, 2, SP], BF16, tag="x2", name="x2_0")
        proj0 = emit_proj_closures(0, oT0, x2_0)
        ln2s0, ln2res0 = emit_ln2_stats_closures(0, x2_0)

        oT1 = otpool.tile([128, EC, SP], DT_A, tag="oT", name="oT1")
        emit_att(1, qkTv1, vT1, oT1, proj0 + ln2s0)

        mu2_0, rstd2_0 = ln2res0[0]
        x21_0 = emit_x21(0, x2_0, mu2_0, rstd2_0)

        # batch 1 out-proj + LN2 (PE work overlapping batch 0's gelu stream)
        x2_1 = x2pool.tile([128, EC, 2, SP], BF16, tag="x2", name="x2_1")
        for cl in emit_proj_closures(1, oT1, x2_1):
            cl()
        ln2s1, ln2res1 = emit_ln2_stats_closures(1, x2_1)
        for cl in ln2s1:
            cl()

        h2_0 = h2pool.tile([128, HC, SP], DT_A, tag="h2", name="h2_0")
        emit_mlp1(0, x21_0, h2_0)

        mu2_1, rstd2_1 = ln2res1[0]
        x21_1 = emit_x21(1, x2_1, mu2_1, rstd2_1)
        h2_1 = h2pool.tile([128, HC, SP], DT_A, tag="h2", name="h2_1")
        emit_mlp1(1, x21_1, h2_1)

        emit_mlp2(0, h2_0, x2_0)
        emit_mlp2(1, h2_1, x2_1)

        for p in reversed(ctxstack):
            p.__exit__(None, None, None)

    return nc


_cached = {}


def _get_program(key):
    if key not in _cached:
        nc = build_program(*key)
        _legalize_waits(nc.m)
        _cached[key] = nc
    return _cached[key]


def _prepare_core_inputs(inputs):
    """Host-side folding + parity packing + sharding."""
    x = np.asarray(inputs["x"], np.float32)
    ln1_g = np.asarray(inputs["ln1_g"], np.float32)
    ln1_b = np.asarray(inputs["ln1_b"], np.float32)
    qkv_w = np.asarray(inputs["qkv_w"], np.float32)
    qkv_b = np.asarray(inputs["qkv_b"], np.float32)
    out_w = np.asarray(inputs["out_w"], np.float32)
    out_b = np.asarray(inputs["out_b"], np.float32)
    ln2_g = np.asarray(inputs["ln2_g"], np.float32)
    ln2_b = np.asarray(inputs["ln2_b"], np.float32)
    w1 = np.asarray(inputs["w1"], np.float32)
    b1 = np.asarray(inputs["b1"], np.float32)
    w2 = np.asarray(inputs["w2"], np.float32)
    b2 = np.asarray(inputs["b2"], np.float32)

    qscale = 1.0 / np.sqrt(D)
    wqkv_eff = ln1_g[:, None] * qkv_w
    vqkv = ln1_b @ qkv_w + qkv_b
    wqkv_eff[:, :E] *= qscale
    vqkv = vqkv.copy()
    vqkv[:E] *= qscale
    w1_eff = ln2_g[:, None] * w1
    vmlp = ln2_b @ w1 + b1

    has_qk_bias = bool(np.any(vqkv[: 2 * E] != 0.0))
    has_v_bias = bool(np.any(vqkv[2 * E:] != 0.0))
    has_out_bias = bool(np.any(out_b != 0.0))
    has_b2 = bool(np.any(b2 != 0.0))
    key = (has_qk_bias, has_v_bias, has_out_bias, has_b2)

    wqkv_c = (wqkv_eff * WS).astype(NPDT)
    wout_c = (out_w * WS).astype(NPDT)
    w1_c = (w1_eff * WS).astype(NPDT)
    w2_c = (w2 * WS).astype(NPDT)

    # masks: slot 0/2 = prev-block (keep q<=k), slot 1/3 = diag (keep k<=q)
    ki = np.arange(128)[:, None]
    qi = np.arange(128)[None, :]
    mprev = (qi <= ki).astype(np.float32)
    mdiag = (ki <= qi).astype(np.float32)
    msk = np.stack([mprev, mdiag, mprev, mdiag], axis=1)     # [128, 4, 128]
    msk = msk.reshape(128, 4 * 128).astype(ml_dtypes.bfloat16)

    # selbc: [8, HP, 128]: OS where h == 2*hp + (p>=64)
    selbc = np.zeros((8, HP, 128), np.float32)
    for hp in range(HP):
        selbc[2 * hp, hp, :64] = OS
        selbc[2 * hp + 1, hp, 64:] = OS
    selbc = selbc.reshape(8, HP * 128).astype(ml_dtypes.bfloat16)

    # x: transpose + halo + parity pack: [B, E, NP, U] -> flat [B, E, NT]
    xT_full = np.ascontiguousarray(x.transpose(0, 2, 1))  # [B, E, L]
    in_maps = []
    for core in range(N_CORES):
        s = core * S
        slab = np.zeros((B, E, S + HALO), np.float32)
        lo = s - HALO
        src_lo = max(lo, 0)
        slab[:, :, src_lo - lo:] = xT_full[:, :, src_lo:s + S]
        xpk = slab.reshape(B, E, U, NP).transpose(0, 1, 3, 2)  # [B, E, NP, U]
        xpk = np.ascontiguousarray(xpk).reshape(B, E, NT)

        # ohsel: [128, 2, H, H]: kind 0 = halo-valid one-hot, kind 1 = ones one-hot
        ohsel = np.zeros((128, 2, H, H), np.float32)
        halo_valid = 0.0 if core == 0 else 1.0
        for h in range(H):
            ohsel[:, 0, h, h] = halo_valid
            ohsel[:, 1, h, h] = 1.0
        ohsel = ohsel.reshape(128, 2 * H * H).astype(ml_dtypes.bfloat16)

        im = {
            "xp": xpk,
            "wqkv": wqkv_c,
            "wout": wout_c,
            "w1": w1_c,
            "w2": w2_c,
            "vmlp": vmlp.astype(np.float32),
            "msk": msk,
            "ohsel": ohsel,
            "selbc": selbc,
        }
        if has_qk_bias:
            im["vqk"] = vqkv[: 2 * E].astype(np.float32)
        if has_v_bias:
            im["vvb"] = vqkv[2 * E:].astype(np.float32)
            im["vhalo"] = np.full(128, halo_valid, np.float32)
        if has_out_bias:
            im["outb"] = out_b.astype(np.float32)
        if has_b2:
            im["b2v"] = b2.astype(np.float32)
        in_maps.append(im)
    return key, in_maps


_last_results = None


def kernel(**inputs) -> np.ndarray:
    global _last_results
    key, in_maps = _prepare_core_inputs(inputs)
    nc = _get_program(key)
    res = run_bass_kernel_spmd(nc, in_maps, core_ids=list(range(N_CORES)))
    _last_results = res
    out = np.empty((B, L, E), np.float32)
    for core in range(N_CORES):
        yp = res.results[core]["yT"]          # [B, E, SP] parity-packed
        yu = yp.reshape(B, E, NP, UQ).transpose(0, 1, 3, 2).reshape(B, E, S)
        out[:, core * S:(core + 1) * S, :] = yu.transpose(0, 2, 1)
    return out
